# revision 22
# baseline (speedup 1.0000x reference)
"""GATv2 AttentionEncoder kernel for Trainium2 (8 NeuronCores, Bass/Tile).

Strategy (sharding_hint: shard by graph):
  - 256 graphs -> 8 cores x 32 graphs. Each core owns a contiguous,
    graph-aligned node slice, padded to NMAXP rows (multiple of 128).
  - Per layer: node-phase matmuls (xl = h@Wl+bl, xr = h@Wr+br) run on the
    local slice; xl is AllGathered (src edges reference any node), xr stays
    local (edges are bucketed by dst core).
  - Edge phase: real edges only (self-loops folded analytically in the
    epilogue), sorted by dst 128-node range. Per range: dma_gather xl rows
    for the range's edge tiles (int16 idx, lo/hi window split for the 32k
    limit). xr values are NOT gathered: a node-major one-hot wT (built via a
    K=1 outer-product broadcast of dst ids + DVE is_equal) permutes the
    range's xr tile into edge order with one PE matmul per 128-edge tile.
    Batched DVE ops compute leaky_relu/att-dot/exp, a batched edge-major
    one-hot w3 (alpha folded) scatters alpha*xl into PSUM via per-tile
    matmuls (plus rhs=ones matmuls for the softmax denominators); the
    self-loop term (exp(att.leaky(xl_i+xr_i)), xl_i) is added on the node
    layout before normalization.
  - Pooling is graph-local (one-hot matmul with 1/cnt folded in), MLP head
    computed per-core on its 32 graphs; host concatenates.
"""

import sys

sys.path.insert(0, "/opt/trn_rl_repo")

import contextlib

import ml_dtypes
import numpy as np

import concourse.bass as bass
import concourse.bacc as bacc
import concourse.mybir as mybir
import concourse.tile as tile
from concourse.bass_utils import run_bass_kernel_spmd

F32 = mybir.dt.float32
BF16 = mybir.dt.bfloat16
I16 = mybir.dt.int16
AX = mybir.AxisListType
OP = mybir.AluOpType
ACTF = mybir.ActivationFunctionType

SLOPE = 0.2


# ----------------------------------------------------------------------------
# Host-side preprocessing
# ----------------------------------------------------------------------------

def _wrap_idx(arr):
    """[n] int array (n % 16 == 0) -> [128, n/16] int16, slot i at
    [i%16, i//16], replicated 8x across partition groups of 16."""
    n = len(arr)
    w = np.ascontiguousarray(arr.reshape(n // 16, 16).T).astype(np.int16)
    return np.tile(w, (8, 1))


def preprocess(inputs, n_cores=8, split=32768):
    x = np.asarray(inputs["x"], np.float32)
    ei = np.asarray(inputs["edge_index"], np.int64)
    batch = np.asarray(inputs["batch"], np.int64)
    N, n_in = x.shape
    G = inputs["_n_graphs"]
    gpc = G // n_cores
    dims = inputs["_dims"]
    ebytes = 2
    # row byte-stride of xl/xr must be a multiple of 256B
    cpads = [(co * ebytes + 255) // 256 * 256 // ebytes for (_, co) in dims]

    cnt = np.bincount(batch, minlength=G)
    gs = np.add.reduceat(cnt, np.arange(0, G, gpc))  # nodes per core
    bounds = np.concatenate([[0], np.cumsum(gs)]).astype(np.int64)
    NMAXP = int((gs.max() + 127) // 128 * 128)
    R = NMAXP // 128
    NFULL = n_cores * NMAXP
    assert NFULL <= split * 2, (NFULL, split)

    # remap node ids into the padded global layout
    newid = np.empty(N, np.int64)
    for r in range(n_cores):
        n0, n1 = bounds[r], bounds[r + 1]
        newid[n0:n1] = NMAXP * r + np.arange(n1 - n0)

    # real edges only; self-loop contributions are computed analytically
    src = newid[ei[0]]
    dst = newid[ei[1]]

    # bucket edges per (core, range, lo/hi); order within a bucket irrelevant
    core_of = dst // NMAXP
    dstl = dst - core_of * NMAXP
    rng_of = dstl // 128
    is_hi = (src >= split).astype(np.int64)
    key = (core_of * R + rng_of) * 2 + is_hi
    order = np.argsort(key, kind="stable")
    src_s, dstl_s, key_s = src[order], dstl[order], key[order]
    uniq, starts = np.unique(key_s, return_index=True)
    starts = list(starts) + [len(key_s)]
    lo_lists = [[None] * R for _ in range(n_cores)]
    hi_lists = [[None] * R for _ in range(n_cores)]
    for i, k in enumerate(uniq):
        e0, e1 = starts[i], starts[i + 1]
        c, rem = divmod(int(k), 2 * R)
        g, h = divmod(rem, 2)
        pair = (src_s[e0:e1], dstl_s[e0:e1])
        (hi_lists if h else lo_lists)[c][g] = pair

    empty = (np.zeros(0, np.int64), np.zeros(0, np.int64))
    TLO = np.zeros(R, np.int64)
    THI = np.zeros(R, np.int64)
    for g in range(R):
        for c in range(n_cores):
            lo = lo_lists[c][g] or empty
            hi = hi_lists[c][g] or empty
            TLO[g] = max(TLO[g], -(-len(lo[0]) // 128))
            THI[g] = max(THI[g], -(-len(hi[0]) // 128))
    T = TLO + THI

    meta = dict(
        n_cores=n_cores, gpc=gpc, G=G, NMAXP=NMAXP, R=R, NFULL=NFULL,
        split=split, TLO=TLO.tolist(), THI=THI.tolist(), T=T.tolist(),
        dims=dims, cpads=cpads, n_in=n_in,
        nhid=inputs["_nhid"], nout=inputs["_nout"],
    )

    # ---- shared const arrays ----
    iota = np.tile(np.arange(128, dtype=np.float32), (128, 1))
    ident = np.eye(128, dtype=np.float32)
    ones = np.ones((128, 1), np.float32).astype(ml_dtypes.bfloat16)
    ones1 = np.ones((1, 128), np.float32).astype(ml_dtypes.bfloat16)
    prow = np.arange(128, dtype=np.float32).reshape(128, 1)

    def bc(v, w):  # broadcast a [w] vector to [128, w]
        return np.tile(np.asarray(v, np.float32).reshape(1, w), (128, 1))

    def padk(w):  # pad leading dim to a multiple of 128
        k = (-(-w.shape[0] // 128)) * 128
        out = np.zeros((k,) + w.shape[1:], np.float32)
        out[: w.shape[0]] = w
        return out

    consts = dict(iota=iota, ident=ident, ones=ones, ones1=ones1, prow=prow,
                  iotab=iota.astype(ml_dtypes.bfloat16),
                  identb=ident.astype(ml_dtypes.bfloat16))
    for l, (ci, co) in enumerate(dims):
        consts[f"wl{l}"] = padk(np.asarray(inputs[f"Wl{l}"], np.float32))
        consts[f"wr{l}"] = padk(np.asarray(inputs[f"Wr{l}"], np.float32))
        consts[f"blb{l}"] = bc(inputs[f"bl{l}"], co)
        consts[f"brb{l}"] = bc(inputs[f"br{l}"], co)
        consts[f"bib{l}"] = bc(inputs[f"bias{l}"], co)
        consts[f"attb{l}"] = bc(inputs[f"att{l}"], co).astype(ml_dtypes.bfloat16)
        # edge path folds leaky's 0.6 factor into att: leaky(x)=0.6(x+(2/3)|x|)
        consts[f"att6b{l}"] = (0.6 * bc(inputs[f"att{l}"], co)).astype(
            ml_dtypes.bfloat16)
    consts["fc1"] = padk(np.asarray(inputs["fc1_W"], np.float32))
    consts["fc2"] = padk(np.asarray(inputs["fc2_W"], np.float32))
    consts["b1b"] = bc(inputs["fc1_b"], meta["nhid"])
    consts["b2b"] = bc(inputs["fc2_b"], meta["nout"])

    rcnt = 1.0 / np.maximum(cnt, 1).astype(np.float64)
    KIN = -(-n_in // 128)

    in_maps = []
    for c in range(n_cores):
        n0, n1 = bounds[c], bounds[c + 1]
        nl = int(n1 - n0)
        xT = np.zeros((KIN * 128, NMAXP), np.float32)
        xT[:n_in, :nl] = x[n0:n1].T
        ilo, ihi, dfb, drb = [], [], [], []
        for g in range(R):
            lo = lo_lists[c][g] or empty
            hi = hi_lists[c][g] or empty
            nlo, nhi = 128 * int(TLO[g]), 128 * int(THI[g])
            sl = np.zeros(nlo, np.int64)
            sl[: len(lo[0])] = lo[0]
            sh = np.zeros(nhi, np.int64)
            sh[: len(hi[0])] = hi[0] - split
            df = np.full(nlo + nhi, -1.0, np.float32)
            df[: len(lo[1])] = lo[1] - g * 128
            df[nlo: nlo + len(hi[1])] = hi[1] - g * 128
            if nlo:
                ilo.append(_wrap_idx(sl))
            if nhi:
                ihi.append(_wrap_idx(sh))
            if nlo + nhi:
                dfb.append(np.ascontiguousarray(df.reshape(-1, 128).T))
                drb.append(df.reshape(1, -1))
        m = dict(
            xT=xT,
            idx_lo=np.concatenate(ilo, 1) if ilo else np.zeros((128, 0), np.int16),
            idx_hi=np.concatenate(ihi, 1) if ihi else np.zeros((128, 0), np.int16),
            dstf=np.concatenate(dfb, 1).astype(ml_dtypes.bfloat16)
            if dfb else np.zeros((128, 0), ml_dtypes.bfloat16),
            dstrow=np.concatenate(drb, 1).astype(ml_dtypes.bfloat16)
            if drb else np.zeros((1, 0), ml_dtypes.bfloat16),
        )
        bf = np.full(NMAXP, -1.0, np.float32)
        rc = np.zeros(NMAXP, np.float32)
        bf[:nl] = (batch[n0:n1] - c * gpc).astype(np.float32)
        rc[:nl] = rcnt[batch[n0:n1]].astype(np.float32)
        m["batchf"] = np.ascontiguousarray(bf.reshape(R, 128).T)
        m["rcntn"] = np.ascontiguousarray(rc.reshape(R, 128).T)
        m.update(consts)
        in_maps.append({k: np.ascontiguousarray(v) for k, v in m.items()})

    return meta, in_maps


# ----------------------------------------------------------------------------
# Bass program
# ----------------------------------------------------------------------------

def build_nc(meta):
    n_cores = meta["n_cores"]
    NMAXP, R, NFULL = meta["NMAXP"], meta["R"], meta["NFULL"]
    split = meta["split"]
    TLO, THI, T = meta["TLO"], meta["THI"], meta["T"]
    dims, cpads = meta["dims"], meta["cpads"]
    n_in, nhid, nout, gpc = meta["n_in"], meta["nhid"], meta["nout"], meta["gpc"]
    EDT = BF16
    KIN = -(-n_in // 128)
    KH = -(-nhid // 128)
    n_layers = len(dims)
    Tmax = max(T)
    CPmax = max(cpads)
    COmax = max(co for _, co in dims)
    nsplit = -(-nout // 512)
    nw = nout // nsplit
    PSP = max(COmax, nhid, nw)  # shared psum tile width (f32, <= 1 bank)
    DCOL = (COmax + 31) // 32 * 32  # aligned denominator column in pv
    assert (PSP + 1) * 4 <= 2048 and DCOL < PSP
    rg = [list(range(n_cores))]

    nc = bacc.Bacc(trn_type="TRN2", num_devices=n_cores)

    def inp(name, shape, dtype=F32):
        return nc.dram_tensor(name, list(shape), dtype, kind="ExternalInput").ap()

    xT = inp("xT", [KIN * 128, NMAXP])
    idx_lo = inp("idx_lo", [128, max(8 * sum(TLO), 1)], I16)
    idx_hi = inp("idx_hi", [128, max(8 * sum(THI), 1)], I16)
    dstf_i = inp("dstf", [128, max(sum(T), 1)], BF16)
    dstrow_i = inp("dstrow", [1, max(128 * sum(T), 1)], BF16)
    batchf_i = inp("batchf", [128, R])
    rcntn_i = inp("rcntn", [128, R])
    iota_i = inp("iota", [128, 128])
    iotab_i = inp("iotab", [128, 128], BF16)
    ident_i = inp("ident", [128, 128])
    identb_i = inp("identb", [128, 128], BF16)
    ones_i = inp("ones", [128, 1], BF16)
    ones1_i = inp("ones1", [1, 128], BF16)
    prow_i = inp("prow", [128, 1])
    w_i = {}
    for l, (ci, co) in enumerate(dims):
        kc = -(-ci // 128)
        w_i[f"wl{l}"] = inp(f"wl{l}", [kc * 128, co])
        w_i[f"wr{l}"] = inp(f"wr{l}", [kc * 128, co])
        for nm in ("blb", "brb", "bib"):
            w_i[f"{nm}{l}"] = inp(f"{nm}{l}", [128, co])
        w_i[f"attb{l}"] = inp(f"attb{l}", [128, co], BF16)
        w_i[f"att6b{l}"] = inp(f"att6b{l}", [128, co], BF16)
    fc1_i = inp("fc1", [KH * 128, nhid])
    fc2_i = inp("fc2", [KH * 128, nout])
    b1b_i = inp("b1b", [128, nhid])
    b2b_i = inp("b2b", [128, nout])
    out_t = nc.dram_tensor("out", [gpc, nout], F32, kind="ExternalOutput").ap()

    with tile.TileContext(nc) as tc, contextlib.ExitStack() as ctx:
        cpool = ctx.enter_context(tc.tile_pool(name="consts", bufs=1))
        sb = ctx.enter_context(tc.tile_pool(name="sb", bufs=2))
        psum = ctx.enter_context(tc.tile_pool(name="ps", bufs=1, space="PSUM"))
        dram = ctx.enter_context(tc.tile_pool(name="dr", bufs=1, space="DRAM"))

        def cload(ap, name, rows=None):
            shape = list(ap.shape) if rows is None else [rows, ap.shape[1]]
            t = cpool.tile(shape, ap.dtype, name=name, tag=name)
            nc.sync.dma_start(out=t[:], in_=ap if rows is None else ap[:rows, :])
            return t

        iota = cload(iota_i, "iota")
        iotab = cload(iotab_i, "iotab")
        ident = cload(ident_i, "ident")
        identb = cload(identb_i, "identb")
        ones = cload(ones_i, "ones")
        ones1 = cload(ones1_i, "ones1")
        prow = cload(prow_i, "prow")
        wt = {}
        for l, (ci, co) in enumerate(dims):
            kc = -(-ci // 128)
            for side in ("wl", "wr"):
                for k in range(kc):
                    nm = f"{side}{l}k{k}"
                    t = cpool.tile([128, co], F32, name=nm, tag=nm)
                    nc.sync.dma_start(
                        out=t[:], in_=w_i[f"{side}{l}"][k * 128:(k + 1) * 128, :])
                    wt[nm] = t
            for nm0 in ("blb", "brb", "bib", "attb", "att6b"):
                wt[f"{nm0}{l}"] = cload(w_i[f"{nm0}{l}"], f"{nm0}{l}")
        fc1c, fc2c = [], []
        for k in range(KH):
            t = cpool.tile([128, nhid], F32, name=f"fc1k{k}", tag=f"fc1k{k}")
            nc.sync.dma_start(out=t[:], in_=fc1_i[k * 128:(k + 1) * 128, :])
            fc1c.append(t)
            t = cpool.tile([128, nout], F32, name=f"fc2k{k}", tag=f"fc2k{k}")
            nc.sync.dma_start(out=t[:], in_=fc2_i[k * 128:(k + 1) * 128, :])
            fc2c.append(t)
        b1b = cload(b1b_i, "b1b")
        b2b = cload(b2b_i, "b2b")
        batchf = cload(batchf_i, "batchf")
        rcntn = cload(rcntn_i, "rcntn")
        # graph topology is layer-invariant: load idx/dst arrays once
        ixloC = cload(idx_lo, "ixloC")
        ixhiC = cload(idx_hi, "ixhiC")
        dstfC = cload(dstf_i, "dstfC")

        # persistent DRAM buffers; AllGather outputs are distinct per layer
        # (a fast core's AG for layer l+1 may write a slow core's output
        # buffer while it still reads layer l's), and Shared for perf.
        xlf_space = "Shared" if n_cores > 4 else "Local"
        xlf = [dram.tile([NFULL, cpads[l]], EDT, name=f"xlf{l}", tag=f"xlf{l}",
                         addr_space=xlf_space) for l in range(n_layers)]
        xl_loc = [dram.tile([NMAXP, cpads[l]], EDT, name=f"xlloc{l}",
                            tag=f"xlloc{l}") for l in range(n_layers)]
        xr_loc = [dram.tile([NMAXP, cpads[l]], EDT, name=f"xrloc{l}",
                            tag=f"xrloc{l}") for l in range(n_layers)]
        hbuf = [dram.tile([NMAXP, dims[l][1]], F32, name=f"h{l}", tag=f"h{l}")
                for l in range(n_layers)]

        reg_cache = {}

        def nreg(v):
            if v not in reg_cache:
                reg_cache[v] = nc.gpsimd.to_reg(v)
            return reg_cache[v]

        # prefix offsets into the per-range packed arrays
        OLO, OHI, ODF = [], [], []
        olo = ohi = odf = 0
        for r in range(R):
            OLO.append(olo); OHI.append(ohi); ODF.append(odf)
            olo += 8 * TLO[r]; ohi += 8 * THI[r]; odf += T[r]

        def node_range(l, r):
            ci, co = dims[l]
            kc = -(-ci // 128)
            hTs = []
            if l == 0:
                for k in range(kc):
                    hT = sb.tile([128, 128], F32, name=f"hT{l}_{r}_{k}",
                                 tag=f"hT{k}")
                    nc.sync.dma_start(
                        out=hT[:],
                        in_=xT[k * 128:(k + 1) * 128, r * 128:(r + 1) * 128])
                    hTs.append(hT)
            else:
                ht = sb.tile([128, ci], F32, name=f"ht{l}_{r}", tag="ht",
                             padded_shape=[128, 128])
                nc.sync.dma_start(
                    out=ht[:], in_=hbuf[l - 1][r * 128:(r + 1) * 128, :])
                pt = psum.tile([ci, 128], F32, name=f"pt{l}_{r}", tag="pt",
                               bufs=2, padded_shape=[128, 512])
                nc.tensor.transpose(out=pt[:], in_=ht[:], identity=ident[:])
                hT = sb.tile([128, 128], F32, name=f"hT{l}_{r}", tag="hT0")
                nc.vector.tensor_copy(out=hT[:ci, :], in_=pt[:])
                hTs.append(hT)
            krows = [128] * kc if l == 0 else [ci]
            pxl = psum.tile([128, co], F32, name=f"pxl{l}_{r}", tag="pv",
                            bufs=2, padded_shape=[128, PSP])
            pxr = psum.tile([128, co], F32, name=f"pxr{l}_{r}", tag="pperm",
                            bufs=2, padded_shape=[128, PSP])
            for k in range(kc):
                nc.tensor.matmul(out=pxl[:], lhsT=hTs[k][:krows[k], :],
                                 rhs=wt[f"wl{l}k{k}"][:krows[k], :],
                                 start=(k == 0), stop=(k == kc - 1))
            for k in range(kc):
                nc.tensor.matmul(out=pxr[:], lhsT=hTs[k][:krows[k], :],
                                 rhs=wt[f"wr{l}k{k}"][:krows[k], :],
                                 start=(k == 0), stop=(k == kc - 1))
            xls = sb.tile([128, co], EDT, name=f"xls{l}_{r}", tag="xls",
                          padded_shape=[128, COmax])
            xrs = sb.tile([128, co], EDT, name=f"xrs{l}_{r}", tag="xrs",
                          padded_shape=[128, COmax])
            nc.vector.tensor_tensor(out=xls[:], in0=pxl[:],
                                    in1=wt[f"blb{l}"][:], op=OP.add)
            nc.vector.tensor_tensor(out=xrs[:], in0=pxr[:],
                                    in1=wt[f"brb{l}"][:], op=OP.add)
            nc.sync.dma_start(out=xl_loc[l][r * 128:(r + 1) * 128, :co],
                              in_=xls[:])
            nc.sync.dma_start(out=xr_loc[l][r * 128:(r + 1) * 128, :co],
                              in_=xrs[:])

        def edge_range(l, r):
            ci, co = dims[l]
            cp = cpads[l]
            tlo, thi, tt = TLO[r], THI[r], T[r]
            olo, ohi, odf = OLO[r], OHI[r], ODF[r]
            ec = tt * 128  # edge slots this range

            # xls/xrs rows for this range (self-loop term + xr permute)
            xlr = sb.tile([128, co], EDT, name=f"xlr{l}_{r}", tag="xlr",
                          padded_shape=[128, COmax])
            nc.sync.dma_start(out=xlr[:],
                              in_=xl_loc[l][r * 128:(r + 1) * 128, :co])
            xrr = sb.tile([128, co], EDT, name=f"xrr{l}_{r}", tag="xrr",
                          padded_shape=[128, COmax])
            nc.sync.dma_start(out=xrr[:],
                              in_=xr_loc[l][r * 128:(r + 1) * 128, :co])

            # self-loop: s_self = exp(att . leaky(xl_i + xr_i))
            mself = sb.tile([128, co], EDT, name=f"ms{l}_{r}", tag="mself",
                            padded_shape=[128, COmax])
            nc.vector.tensor_tensor(out=mself[:], in0=xlr[:], in1=xrr[:],
                                    op=OP.add)
            # leaky_relu(x) = max(0.2x, x) in one DVE op
            nc.vector.scalar_tensor_tensor(
                out=mself[:], in0=mself[:], scalar=SLOPE, in1=mself[:],
                op0=OP.mult, op1=OP.max)
            nc.vector.tensor_tensor(out=mself[:], in0=mself[:],
                                    in1=wt[f"attb{l}"][:], op=OP.mult)
            lgs = sb.tile([128, 1], F32, name=f"lgs{l}_{r}", tag="lgs")
            nc.vector.tensor_reduce(out=lgs[:], in_=mself[:], axis=AX.X,
                                    op=OP.add)
            sself = sb.tile([128, 1], F32, name=f"ss{l}_{r}", tag="ss")
            nc.scalar.activation(out=sself[:], in_=lgs[:], func=ACTF.Exp)

            if tt == 0:
                # no incoming edges: softmax is all on the self-loop
                hsb = sb.tile([128, co], F32, name=f"hsb{l}_{r}", tag="hsb",
                              padded_shape=[128, COmax])
                nc.vector.tensor_tensor(out=hsb[:], in0=xlr[:],
                                        in1=wt[f"bib{l}"][:], op=OP.add)
                hsb2 = sb.tile([128, co], F32, name=f"hsb2{l}_{r}",
                               tag="hsb2", padded_shape=[128, COmax])
                nc.scalar.activation(out=hsb2[:], in_=hsb[:],
                                     func=ACTF.Relu)
                nc.sync.dma_start(out=hbuf[l][r * 128:(r + 1) * 128, :],
                                  in_=hsb2[:])
                return

            # ---- gather xl rows for this range's edges ----
            gxl = sb.tile([128, tt * cp], EDT, name=f"gxl{l}_{r}",
                          tag="gxl", bufs=3, padded_shape=[128, Tmax * CPmax])
            g3l = gxl[:].rearrange("p (t c) -> p t c", c=cp)
            dsr = sb.tile([1, ec], BF16, name=f"dsr{l}_{r}", tag="dsr",
                          padded_shape=[1, Tmax * 128])
            nc.sync.dma_start(out=dsr[:],
                              in_=dstrow_i[:, 128 * odf:128 * odf + ec])
            # HW limit: <= 1024 idxs (8 tiles) per dma_gather
            GCH = 8

            def chunked_gather(g3, t0, tn, src_ap, ix, ix0):
                for c0 in range(0, tn, GCH):
                    cn = min(GCH, tn - c0)
                    nc.gpsimd.dma_gather(
                        out_ap=g3[:, t0 + c0:t0 + c0 + cn, :],
                        in_ap=src_ap,
                        idxs_ap=ix[:, ix0 + 8 * c0:ix0 + 8 * (c0 + cn)],
                        num_idxs=128 * cn, num_idxs_reg=nreg(128 * cn),
                        elem_size=cp)

            if tlo:
                chunked_gather(g3l, 0, tlo, xlf[l][0:min(split, NFULL), :],
                               ixloC, olo)
            if thi:
                chunked_gather(g3l, tlo, thi, xlf[l][split:NFULL, :],
                               ixhiC, ohi)

            # ---- node-major one-hot wT[p, e] = (dst_e == p) ----
            wT = sb.tile([128, ec], EDT, name=f"wT{l}_{r}", tag="wT",
                         padded_shape=[128, Tmax * 128])
            NCH = 512
            for c0 in range(0, ec, NCH):
                cw = min(NCH, ec - c0)
                pdst = psum.tile([128, cw], F32, name=f"pb{l}_{r}_{c0}",
                                 tag="pt", bufs=2,
                                 padded_shape=[128, NCH])
                nc.tensor.matmul(out=pdst[:], lhsT=ones1[:],
                                 rhs=dsr[:, c0:c0 + cw], start=True,
                                 stop=True)
                nc.vector.tensor_tensor(
                    out=wT[:, c0:c0 + cw], in0=pdst[:],
                    in1=prow[:].to_broadcast([128, cw]), op=OP.is_equal)
            wT3 = wT[:].rearrange("p (t c) -> p t c", c=128)

            # ---- permute xr into edge order; m = xl_src + xr_dst ----
            mm = sb.tile([128, tt * cp], EDT, name=f"mm{l}_{r}", tag="gxr",
                         padded_shape=[128, Tmax * CPmax])
            m3 = mm[:].rearrange("p (t c) -> p t c", c=cp)
            for t in range(tt):
                pperm = psum.tile([128, co], F32, name=f"pp{l}_{r}_{t}",
                                  tag="pperm", bufs=2,
                                  padded_shape=[128, PSP])
                nc.tensor.matmul(out=pperm[:], lhsT=wT3[:, t, :],
                                 rhs=xrr[:], start=True, stop=False)
                nc.tensor.matmul(out=pperm[:], lhsT=identb[:],
                                 rhs=g3l[:, t, :co], start=False,
                                 stop=True)
                # leaky(x) = 0.6(x + (2/3)|x|); 0.6 folded into att6b
                nc.scalar.activation(out=m3[:, t, :co], in_=pperm[:],
                                     func=ACTF.Abs)
                nc.vector.scalar_tensor_tensor(
                    out=m3[:, t, :co], in0=m3[:, t, :co],
                    scalar=2.0 / 3.0, in1=pperm[:],
                    op0=OP.mult, op1=OP.add)

            v3 = m3[:, :, :co]
            att3 = wt[f"att6b{l}"][:, None, :].to_broadcast([128, tt, co])
            nc.vector.tensor_tensor(out=v3, in0=v3, in1=att3, op=OP.mult)
            logits = sb.tile([128, tt], F32, name=f"lg{l}_{r}", tag="lg",
                             padded_shape=[128, Tmax])
            nc.vector.tensor_reduce(out=logits[:], in_=v3, axis=AX.X,
                                    op=OP.add)
            s = sb.tile([128, tt], BF16, name=f"s{l}_{r}", tag="s",
                        padded_shape=[128, Tmax])
            nc.scalar.activation(out=s[:], in_=logits[:], func=ACTF.Exp)

            # ---- edge-major one-hot w3[e, t, q] = (dst_e==q) * s_e ----
            w3t = sb.tile([128, ec], EDT, name=f"w3{l}_{r}", tag="w3",
                          padded_shape=[128, Tmax * 128])
            w3 = w3t[:].rearrange("p (t c) -> p t c", c=128)
            dst3 = dstfC[:, odf:odf + tt][:, :, None].to_broadcast(
                [128, tt, 128])
            iot3 = iotab[:][:, None, :].to_broadcast([128, tt, 128])
            s3 = s[:][:, :, None].to_broadcast([128, tt, 128])
            nc.vector.tensor_tensor(out=w3, in0=dst3, in1=iot3,
                                    op=OP.is_equal)
            nc.vector.tensor_tensor(out=w3, in0=w3, in1=s3, op=OP.mult)

            # ---- scatter: pv[q,:] += s_e*xl_e ; pden[q] += s_e ----
            pv = psum.tile([128, co], F32, name=f"pv{l}_{r}", tag="pv",
                           bufs=2, padded_shape=[128, PSP])
            pden = psum.tile([128, 1], F32, name=f"pn{l}_{r}", tag="pden",
                             bufs=2)
            for t in range(tt):
                nc.tensor.matmul(out=pv[:], lhsT=w3[:, t, :],
                                 rhs=g3l[:, t, :co],
                                 start=(t == 0), stop=(t == tt - 1))
                nc.tensor.matmul(out=pden[:], lhsT=w3[:, t, :],
                                 rhs=ones[:],
                                 start=(t == 0), stop=(t == tt - 1))

            # ---- epilogue: add self-loop, normalize, bias, relu ----
            num = sb.tile([128, co], F32, name=f"nm{l}_{r}", tag="num",
                          padded_shape=[128, COmax])
            nc.vector.scalar_tensor_tensor(
                out=num[:], in0=xlr[:], scalar=sself[:, :],
                in1=pv[:], op0=OP.mult, op1=OP.add)
            den = sb.tile([128, 1], F32, name=f"den{l}_{r}", tag="den")
            nc.vector.tensor_tensor(out=den[:], in0=pden[:],
                                    in1=sself[:], op=OP.add)
            rden = sb.tile([128, 1], F32, name=f"rden{l}_{r}", tag="rden")
            nc.vector.reciprocal(out=rden[:], in_=den[:])
            hsb = sb.tile([128, co], F32, name=f"hsb{l}_{r}", tag="hsb",
                          padded_shape=[128, COmax])
            nc.vector.scalar_tensor_tensor(
                out=hsb[:], in0=num[:], scalar=rden[:, :],
                in1=wt[f"bib{l}"][:], op0=OP.mult, op1=OP.add)
            hsb2 = sb.tile([128, co], F32, name=f"hsb2{l}_{r}", tag="hsb2",
                           padded_shape=[128, COmax])
            nc.scalar.activation(out=hsb2[:], in_=hsb[:], func=ACTF.Relu)
            nc.sync.dma_start(out=hbuf[l][r * 128:(r + 1) * 128, :],
                              in_=hsb2[:])

        # ---------------- layers ----------------
        # node(l+1) is interleaved into edge(l) so the AllGather for l+1
        # fires right after the last edge range of layer l.
        for r in range(R):
            node_range(0, r)
        for l in range(n_layers):
            nc.gpsimd.collective_compute(
                "AllGather", OP.bypass, replica_groups=rg,
                ins=[xl_loc[l][:, :]], outs=[xlf[l][:, :]])
            for r in range(R):
                edge_range(l, r)
                if l + 1 < n_layers:
                    node_range(l + 1, r)

        # ---------------- pooling ----------------
        pg = psum.tile([128, nhid], F32, name="pg", tag="pv", bufs=2,
                       padded_shape=[128, PSP])
        for r in range(R):
            h3t = sb.tile([128, nhid], F32, name=f"h3t{r}", tag="h3t")
            nc.sync.dma_start(out=h3t[:], in_=hbuf[-1][r * 128:(r + 1) * 128, :])
            wp = sb.tile([128, 128], F32, name=f"wp{r}", tag="wpool")
            nc.vector.scalar_tensor_tensor(
                out=wp[:], in0=iota[:], scalar=batchf[:, r:r + 1],
                in1=rcntn[:, r:r + 1].to_broadcast([128, 128]),
                op0=OP.is_equal, op1=OP.mult)
            nc.tensor.matmul(out=pg[:], lhsT=wp[:], rhs=h3t[:],
                             start=(r == 0), stop=(r == R - 1))
        gsb = sb.tile([128, nhid], F32, name="gsb", tag="gsb")
        nc.vector.tensor_copy(out=gsb[:], in_=pg[:])

        # ---------------- head ----------------
        gTs = []
        for k in range(KH):
            cw = min(128, nhid - k * 128)
            ptk = psum.tile([cw, 128], F32, name=f"ptg{k}", tag="pt", bufs=2,
                            padded_shape=[128, 512])
            nc.tensor.transpose(out=ptk[:], in_=gsb[:, k * 128:k * 128 + cw],
                                identity=ident[:])
            gT = sb.tile([128, gpc], F32, name=f"gT{k}", tag=f"gT{k}")
            nc.vector.tensor_copy(out=gT[:cw, :], in_=ptk[:cw, :gpc])
            gTs.append((gT, cw))
        pz = psum.tile([gpc, nhid], F32, name="pz", tag="pv", bufs=2,
                       padded_shape=[128, PSP])
        for k in range(KH):
            gT, cw = gTs[k]
            nc.tensor.matmul(out=pz[:], lhsT=gT[:cw, :], rhs=fc1c[k][:cw, :],
                             start=(k == 0), stop=(k == KH - 1))
        zsb = sb.tile([gpc, nhid], F32, name="zsb", tag="zsb")
        nc.vector.tensor_tensor(out=zsb[:], in0=pz[:], in1=b1b[:gpc, :],
                                op=OP.add)
        zsb2 = sb.tile([gpc, nhid], F32, name="zsb2", tag="zsb2")
        nc.scalar.activation(out=zsb2[:], in_=zsb[:], func=ACTF.Relu)
        zTs = []
        for k in range(KH):
            cw = min(128, nhid - k * 128)
            ptk = psum.tile([cw, gpc], F32, name=f"ptz{k}", tag="pt", bufs=2,
                            padded_shape=[128, 512])
            nc.tensor.transpose(out=ptk[:], in_=zsb2[:, k * 128:k * 128 + cw],
                                identity=ident[:gpc, :gpc])
            zT = sb.tile([128, gpc], F32, name=f"zT{k}", tag=f"zT{k}")
            nc.vector.tensor_copy(out=zT[:cw, :], in_=ptk[:cw, :])
            zTs.append((zT, cw))
        osb = sb.tile([gpc, nout], F32, name="osb", tag="osb")
        for j in range(nsplit):
            po = psum.tile([gpc, nw], F32, name=f"po{j}", tag="pperm", bufs=2,
                           padded_shape=[128, PSP])
            for k in range(KH):
                zT, cw = zTs[k]
                nc.tensor.matmul(out=po[:], lhsT=zT[:cw, :],
                                 rhs=fc2c[k][:cw, j * nw:(j + 1) * nw],
                                 start=(k == 0), stop=(k == KH - 1))
            nc.vector.tensor_tensor(out=osb[:, j * nw:(j + 1) * nw], in0=po[:],
                                    in1=b2b[:gpc, j * nw:(j + 1) * nw],
                                    op=OP.add)
        nc.sync.dma_start(out=out_t[:, :], in_=osb[:])

    nc.compile()
    return nc


# ----------------------------------------------------------------------------
# Entry point
# ----------------------------------------------------------------------------

def _augment(inputs):
    inputs = dict(inputs)
    inputs["_dims"] = [(300, 128), (128, 128), (128, 128), (128, 300)]
    inputs["_nhid"] = 300
    inputs["_nout"] = 768
    inputs["_n_graphs"] = 256
    return inputs


def run(inputs, trace=False, n_cores=8):
    inputs = _augment(inputs)
    meta, in_maps = preprocess(inputs, n_cores=n_cores)
    nc = build_nc(meta)
    res = run_bass_kernel_spmd(nc, in_maps, core_ids=list(range(n_cores)),
                               trace=trace)
    out = np.concatenate([r["out"] for r in res.results], axis=0)
    return out, res


def kernel(**inputs):
    out, _ = run(inputs, trace=False)
    return out


# revision 23
# speedup vs baseline: 1.2645x; 1.2645x over previous
"""GATv2 AttentionEncoder kernel for Trainium2 (8 NeuronCores, Bass/Tile).

Strategy (sharding_hint: shard by graph):
  - 256 graphs -> 8 cores x 32 graphs. Each core owns a contiguous,
    graph-aligned node slice, padded to NMAXP rows (multiple of 128).
  - Per layer: node-phase matmuls (xl = h@Wl+bl, xr = h@Wr+br) run on the
    local slice; xl is AllGathered (src edges reference any node), xr stays
    local (edges are bucketed by dst core).
  - Edge phase: real edges only (self-loops folded analytically in the
    epilogue), sorted by dst 128-node range. Per range: dma_gather xl rows
    for the range's edge tiles (int16 idx, lo/hi window split for the 32k
    limit). xr values are NOT gathered: a node-major one-hot wT (built via a
    K=1 outer-product broadcast of dst ids + DVE is_equal) permutes the
    range's xr tile into edge order with one PE matmul per 128-edge tile.
    Batched DVE ops compute leaky_relu/att-dot/exp, a batched edge-major
    one-hot w3 (alpha folded) scatters alpha*xl into PSUM via per-tile
    matmuls (plus rhs=ones matmuls for the softmax denominators); the
    self-loop term (exp(att.leaky(xl_i+xr_i)), xl_i) is added on the node
    layout before normalization.
  - Pooling is graph-local (one-hot matmul with 1/cnt folded in), MLP head
    computed per-core on its 32 graphs; host concatenates.
"""

import sys

sys.path.insert(0, "/opt/trn_rl_repo")

import contextlib

import ml_dtypes
import numpy as np

import concourse.bass as bass
import concourse.bacc as bacc
import concourse.mybir as mybir
import concourse.tile as tile
from concourse.bass_utils import run_bass_kernel_spmd

F32 = mybir.dt.float32
BF16 = mybir.dt.bfloat16
I16 = mybir.dt.int16
AX = mybir.AxisListType
OP = mybir.AluOpType
ACTF = mybir.ActivationFunctionType

SLOPE = 0.2


# ----------------------------------------------------------------------------
# Host-side preprocessing
# ----------------------------------------------------------------------------

def _wrap_idx(arr):
    """[n] int array (n % 16 == 0) -> [128, n/16] int16, slot i at
    [i%16, i//16], replicated 8x across partition groups of 16."""
    n = len(arr)
    w = np.ascontiguousarray(arr.reshape(n // 16, 16).T).astype(np.int16)
    return np.tile(w, (8, 1))


def preprocess(inputs, n_cores=8, split=32768):
    x = np.asarray(inputs["x"], np.float32)
    ei = np.asarray(inputs["edge_index"], np.int64)
    batch = np.asarray(inputs["batch"], np.int64)
    N, n_in = x.shape
    G = inputs["_n_graphs"]
    gpc = G // n_cores
    dims = inputs["_dims"]
    ebytes = 2
    # row byte-stride of xl/xr must be a multiple of 256B
    cpads = [(co * ebytes + 255) // 256 * 256 // ebytes for (_, co) in dims]

    cnt = np.bincount(batch, minlength=G)
    gs = np.add.reduceat(cnt, np.arange(0, G, gpc))  # nodes per core
    bounds = np.concatenate([[0], np.cumsum(gs)]).astype(np.int64)
    NMAXP = int((gs.max() + 127) // 128 * 128)
    R = NMAXP // 128
    NFULL = n_cores * NMAXP
    assert NFULL <= split * 2, (NFULL, split)

    # remap node ids into the padded global layout
    newid = np.empty(N, np.int64)
    for r in range(n_cores):
        n0, n1 = bounds[r], bounds[r + 1]
        newid[n0:n1] = NMAXP * r + np.arange(n1 - n0)

    # real edges only; self-loop contributions are computed analytically
    src = newid[ei[0]]
    dst = newid[ei[1]]

    # bucket edges per (core, range, lo/hi); order within a bucket irrelevant
    core_of = dst // NMAXP
    dstl = dst - core_of * NMAXP
    rng_of = dstl // 128
    is_hi = (src >= split).astype(np.int64)
    key = (core_of * R + rng_of) * 2 + is_hi
    order = np.argsort(key, kind="stable")
    src_s, dstl_s, key_s = src[order], dstl[order], key[order]
    uniq, starts = np.unique(key_s, return_index=True)
    starts = list(starts) + [len(key_s)]
    lo_lists = [[None] * R for _ in range(n_cores)]
    hi_lists = [[None] * R for _ in range(n_cores)]
    for i, k in enumerate(uniq):
        e0, e1 = starts[i], starts[i + 1]
        c, rem = divmod(int(k), 2 * R)
        g, h = divmod(rem, 2)
        pair = (src_s[e0:e1], dstl_s[e0:e1])
        (hi_lists if h else lo_lists)[c][g] = pair

    empty = (np.zeros(0, np.int64), np.zeros(0, np.int64))
    TLO = np.zeros(R, np.int64)
    THI = np.zeros(R, np.int64)
    for g in range(R):
        for c in range(n_cores):
            lo = lo_lists[c][g] or empty
            hi = hi_lists[c][g] or empty
            TLO[g] = max(TLO[g], -(-len(lo[0]) // 128))
            THI[g] = max(THI[g], -(-len(hi[0]) // 128))
    T = TLO + THI

    meta = dict(
        n_cores=n_cores, gpc=gpc, G=G, NMAXP=NMAXP, R=R, NFULL=NFULL,
        split=split, TLO=TLO.tolist(), THI=THI.tolist(), T=T.tolist(),
        dims=dims, cpads=cpads, n_in=n_in,
        nhid=inputs["_nhid"], nout=inputs["_nout"],
    )

    # ---- shared const arrays ----
    iota = np.tile(np.arange(128, dtype=np.float32), (128, 1))
    ident = np.eye(128, dtype=np.float32)
    ones = np.ones((128, 1), np.float32).astype(ml_dtypes.bfloat16)
    ones1 = np.ones((1, 128), np.float32).astype(ml_dtypes.bfloat16)
    prow = np.arange(128, dtype=np.float32).reshape(128, 1)

    def bc(v, w):  # broadcast a [w] vector to [128, w]
        return np.tile(np.asarray(v, np.float32).reshape(1, w), (128, 1))

    def padk(w):  # pad leading dim to a multiple of 128
        k = (-(-w.shape[0] // 128)) * 128
        out = np.zeros((k,) + w.shape[1:], np.float32)
        out[: w.shape[0]] = w
        return out

    consts = dict(iota=iota, ident=ident, ones=ones, ones1=ones1, prow=prow,
                  iotab=iota.astype(ml_dtypes.bfloat16),
                  identb=ident.astype(ml_dtypes.bfloat16))
    for l, (ci, co) in enumerate(dims):
        consts[f"wl{l}"] = padk(np.asarray(inputs[f"Wl{l}"], np.float32))
        consts[f"wr{l}"] = padk(np.asarray(inputs[f"Wr{l}"], np.float32))
        consts[f"blb{l}"] = bc(inputs[f"bl{l}"], co)
        consts[f"brb{l}"] = bc(inputs[f"br{l}"], co)
        consts[f"bib{l}"] = bc(inputs[f"bias{l}"], co)
        consts[f"attb{l}"] = bc(inputs[f"att{l}"], co).astype(ml_dtypes.bfloat16)
        # edge path folds leaky's 0.6 factor into att: leaky(x)=0.6(x+(2/3)|x|)
        consts[f"att6b{l}"] = (0.6 * bc(inputs[f"att{l}"], co)).astype(
            ml_dtypes.bfloat16)
    consts["fc1"] = padk(np.asarray(inputs["fc1_W"], np.float32))
    consts["fc2"] = padk(np.asarray(inputs["fc2_W"], np.float32))
    consts["b1b"] = bc(inputs["fc1_b"], meta["nhid"])
    consts["b2b"] = bc(inputs["fc2_b"], meta["nout"])

    rcnt = 1.0 / np.maximum(cnt, 1).astype(np.float64)
    KIN = -(-n_in // 128)

    in_maps = []
    for c in range(n_cores):
        n0, n1 = bounds[c], bounds[c + 1]
        nl = int(n1 - n0)
        xT = np.zeros((KIN * 128, NMAXP), np.float32)
        xT[:n_in, :nl] = x[n0:n1].T
        ilo, ihi, dfb, drb = [], [], [], []
        for g in range(R):
            lo = lo_lists[c][g] or empty
            hi = hi_lists[c][g] or empty
            nlo, nhi = 128 * int(TLO[g]), 128 * int(THI[g])
            sl = np.zeros(nlo, np.int64)
            sl[: len(lo[0])] = lo[0]
            sh = np.zeros(nhi, np.int64)
            sh[: len(hi[0])] = hi[0] - split
            df = np.full(nlo + nhi, -1.0, np.float32)
            df[: len(lo[1])] = lo[1] - g * 128
            df[nlo: nlo + len(hi[1])] = hi[1] - g * 128
            if nlo:
                ilo.append(_wrap_idx(sl))
            if nhi:
                ihi.append(_wrap_idx(sh))
            if nlo + nhi:
                dfb.append(np.ascontiguousarray(df.reshape(-1, 128).T))
                drb.append(df.reshape(1, -1))
        m = dict(
            xT=xT,
            idx_lo=np.concatenate(ilo, 1) if ilo else np.zeros((128, 0), np.int16),
            idx_hi=np.concatenate(ihi, 1) if ihi else np.zeros((128, 0), np.int16),
            dstf=np.concatenate(dfb, 1).astype(ml_dtypes.bfloat16)
            if dfb else np.zeros((128, 0), ml_dtypes.bfloat16),
            dstrow=np.concatenate(drb, 1).astype(ml_dtypes.bfloat16)
            if drb else np.zeros((1, 0), ml_dtypes.bfloat16),
        )
        bf = np.full(NMAXP, -1.0, np.float32)
        rc = np.zeros(NMAXP, np.float32)
        bf[:nl] = (batch[n0:n1] - c * gpc).astype(np.float32)
        rc[:nl] = rcnt[batch[n0:n1]].astype(np.float32)
        m["batchf"] = np.ascontiguousarray(bf.reshape(R, 128).T)
        m["rcntn"] = np.ascontiguousarray(rc.reshape(R, 128).T)
        m.update(consts)
        in_maps.append({k: np.ascontiguousarray(v) for k, v in m.items()})

    return meta, in_maps


# ----------------------------------------------------------------------------
# Bass program
# ----------------------------------------------------------------------------

def build_nc(meta):
    n_cores = meta["n_cores"]
    NMAXP, R, NFULL = meta["NMAXP"], meta["R"], meta["NFULL"]
    split = meta["split"]
    TLO, THI, T = meta["TLO"], meta["THI"], meta["T"]
    dims, cpads = meta["dims"], meta["cpads"]
    n_in, nhid, nout, gpc = meta["n_in"], meta["nhid"], meta["nout"], meta["gpc"]
    EDT = BF16
    KIN = -(-n_in // 128)
    KH = -(-nhid // 128)
    n_layers = len(dims)
    Tmax = max(T)
    CPmax = max(cpads)
    COmax = max(co for _, co in dims)
    nsplit = -(-nout // 512)
    nw = nout // nsplit
    PSP = max(COmax, nhid, nw)  # shared psum tile width (f32, <= 1 bank)
    DCOL = (COmax + 31) // 32 * 32  # aligned denominator column in pv
    assert (PSP + 1) * 4 <= 2048 and DCOL < PSP
    rg = [list(range(n_cores))]

    nc = bacc.Bacc(trn_type="TRN2", num_devices=n_cores)

    def inp(name, shape, dtype=F32):
        return nc.dram_tensor(name, list(shape), dtype, kind="ExternalInput").ap()

    xT = inp("xT", [KIN * 128, NMAXP])
    idx_lo = inp("idx_lo", [128, max(8 * sum(TLO), 1)], I16)
    idx_hi = inp("idx_hi", [128, max(8 * sum(THI), 1)], I16)
    dstf_i = inp("dstf", [128, max(sum(T), 1)], BF16)
    dstrow_i = inp("dstrow", [1, max(128 * sum(T), 1)], BF16)
    batchf_i = inp("batchf", [128, R])
    rcntn_i = inp("rcntn", [128, R])
    iota_i = inp("iota", [128, 128])
    iotab_i = inp("iotab", [128, 128], BF16)
    ident_i = inp("ident", [128, 128])
    identb_i = inp("identb", [128, 128], BF16)
    ones_i = inp("ones", [128, 1], BF16)
    ones1_i = inp("ones1", [1, 128], BF16)
    prow_i = inp("prow", [128, 1])
    w_i = {}
    for l, (ci, co) in enumerate(dims):
        kc = -(-ci // 128)
        w_i[f"wl{l}"] = inp(f"wl{l}", [kc * 128, co])
        w_i[f"wr{l}"] = inp(f"wr{l}", [kc * 128, co])
        for nm in ("blb", "brb", "bib"):
            w_i[f"{nm}{l}"] = inp(f"{nm}{l}", [128, co])
        w_i[f"attb{l}"] = inp(f"attb{l}", [128, co], BF16)
        w_i[f"att6b{l}"] = inp(f"att6b{l}", [128, co], BF16)
    fc1_i = inp("fc1", [KH * 128, nhid])
    fc2_i = inp("fc2", [KH * 128, nout])
    b1b_i = inp("b1b", [128, nhid])
    b2b_i = inp("b2b", [128, nout])
    out_t = nc.dram_tensor("out", [gpc, nout], F32, kind="ExternalOutput").ap()

    with tile.TileContext(nc) as tc, contextlib.ExitStack() as ctx:
        cpool = ctx.enter_context(tc.tile_pool(name="consts", bufs=1))
        sb = ctx.enter_context(tc.tile_pool(name="sb", bufs=2))
        psum = ctx.enter_context(tc.tile_pool(name="ps", bufs=1, space="PSUM"))
        dram = ctx.enter_context(tc.tile_pool(name="dr", bufs=1, space="DRAM"))

        def cload(ap, name, rows=None):
            shape = list(ap.shape) if rows is None else [rows, ap.shape[1]]
            t = cpool.tile(shape, ap.dtype, name=name, tag=name)
            nc.sync.dma_start(out=t[:], in_=ap if rows is None else ap[:rows, :])
            return t

        iota = cload(iota_i, "iota")
        iotab = cload(iotab_i, "iotab")
        ident = cload(ident_i, "ident")
        identb = cload(identb_i, "identb")
        ones = cload(ones_i, "ones")
        ones1 = cload(ones1_i, "ones1")
        prow = cload(prow_i, "prow")
        wt = {}
        for l, (ci, co) in enumerate(dims):
            kc = -(-ci // 128)
            for side in ("wl", "wr"):
                for k in range(kc):
                    nm = f"{side}{l}k{k}"
                    t = cpool.tile([128, co], F32, name=nm, tag=nm)
                    nc.sync.dma_start(
                        out=t[:], in_=w_i[f"{side}{l}"][k * 128:(k + 1) * 128, :])
                    wt[nm] = t
            for nm0 in ("blb", "brb", "bib", "attb", "att6b"):
                wt[f"{nm0}{l}"] = cload(w_i[f"{nm0}{l}"], f"{nm0}{l}")
        fc1c, fc2c = [], []
        for k in range(KH):
            t = cpool.tile([128, nhid], F32, name=f"fc1k{k}", tag=f"fc1k{k}")
            nc.sync.dma_start(out=t[:], in_=fc1_i[k * 128:(k + 1) * 128, :])
            fc1c.append(t)
            t = cpool.tile([128, nout], F32, name=f"fc2k{k}", tag=f"fc2k{k}")
            nc.sync.dma_start(out=t[:], in_=fc2_i[k * 128:(k + 1) * 128, :])
            fc2c.append(t)
        b1b = cload(b1b_i, "b1b")
        b2b = cload(b2b_i, "b2b")
        batchf = cload(batchf_i, "batchf")
        rcntn = cload(rcntn_i, "rcntn")
        # graph topology is layer-invariant: load idx/dst arrays once
        ixloC = cload(idx_lo, "ixloC")
        ixhiC = cload(idx_hi, "ixhiC")
        dstfC = cload(dstf_i, "dstfC")

        # persistent DRAM buffers; AllGather outputs are distinct per layer
        # (a fast core's AG for layer l+1 may write a slow core's output
        # buffer while it still reads layer l's), and Shared for perf.
        xlf_space = "Shared" if n_cores > 4 else "Local"
        xlf = [dram.tile([NFULL, cpads[l]], EDT, name=f"xlf{l}", tag=f"xlf{l}",
                         addr_space=xlf_space) for l in range(n_layers)]
        xl_loc = [dram.tile([NMAXP, cpads[l]], EDT, name=f"xlloc{l}",
                            tag=f"xlloc{l}") for l in range(n_layers)]
        xr_loc = [dram.tile([NMAXP, cpads[l]], EDT, name=f"xrloc{l}",
                            tag=f"xrloc{l}") for l in range(n_layers)]
        hbuf = [dram.tile([NMAXP, dims[l][1]], F32, name=f"h{l}", tag=f"h{l}")
                for l in range(n_layers)]

        reg_cache = {}

        def nreg(v):
            if v not in reg_cache:
                reg_cache[v] = nc.gpsimd.to_reg(v)
            return reg_cache[v]

        # prefix offsets into the per-range packed arrays
        OLO, OHI, ODF = [], [], []
        olo = ohi = odf = 0
        for r in range(R):
            OLO.append(olo); OHI.append(ohi); ODF.append(odf)
            olo += 8 * TLO[r]; ohi += 8 * THI[r]; odf += T[r]

        def node_range(l, r):
            ci, co = dims[l]
            kc = -(-ci // 128)
            hTs = []
            if l == 0:
                for k in range(kc):
                    hT = sb.tile([128, 128], F32, name=f"hT{l}_{r}_{k}",
                                 tag=f"hT{k}")
                    nc.sync.dma_start(
                        out=hT[:],
                        in_=xT[k * 128:(k + 1) * 128, r * 128:(r + 1) * 128])
                    hTs.append(hT)
            else:
                ht = sb.tile([128, ci], F32, name=f"ht{l}_{r}", tag="ht",
                             padded_shape=[128, 128])
                nc.sync.dma_start(
                    out=ht[:], in_=hbuf[l - 1][r * 128:(r + 1) * 128, :])
                pt = psum.tile([ci, 128], F32, name=f"pt{l}_{r}", tag="pt",
                               bufs=2, padded_shape=[128, 512])
                nc.tensor.transpose(out=pt[:], in_=ht[:], identity=ident[:])
                hT = sb.tile([128, 128], F32, name=f"hT{l}_{r}", tag="hT0")
                nc.vector.tensor_copy(out=hT[:ci, :], in_=pt[:])
                hTs.append(hT)
            krows = [128] * kc if l == 0 else [ci]
            pxl = psum.tile([128, co], F32, name=f"pxl{l}_{r}", tag="pv",
                            bufs=2, padded_shape=[128, PSP])
            pxr = psum.tile([128, co], F32, name=f"pxr{l}_{r}", tag="pperm",
                            bufs=2, padded_shape=[128, PSP])
            for k in range(kc):
                nc.tensor.matmul(out=pxl[:], lhsT=hTs[k][:krows[k], :],
                                 rhs=wt[f"wl{l}k{k}"][:krows[k], :],
                                 start=(k == 0), stop=(k == kc - 1))
            for k in range(kc):
                nc.tensor.matmul(out=pxr[:], lhsT=hTs[k][:krows[k], :],
                                 rhs=wt[f"wr{l}k{k}"][:krows[k], :],
                                 start=(k == 0), stop=(k == kc - 1))
            xls = sb.tile([128, co], EDT, name=f"xls{l}_{r}", tag="xls",
                          padded_shape=[128, COmax])
            xrs = sb.tile([128, co], EDT, name=f"xrs{l}_{r}", tag="xrs",
                          padded_shape=[128, COmax])
            nc.vector.tensor_tensor(out=xls[:], in0=pxl[:],
                                    in1=wt[f"blb{l}"][:], op=OP.add)
            nc.vector.tensor_tensor(out=xrs[:], in0=pxr[:],
                                    in1=wt[f"brb{l}"][:], op=OP.add)
            nc.sync.dma_start(out=xl_loc[l][r * 128:(r + 1) * 128, :co],
                              in_=xls[:])
            nc.sync.dma_start(out=xr_loc[l][r * 128:(r + 1) * 128, :co],
                              in_=xrs[:])

        def edge_range(l, r):
            ci, co = dims[l]
            cp = cpads[l]
            tlo, thi, tt = TLO[r], THI[r], T[r]
            olo, ohi, odf = OLO[r], OHI[r], ODF[r]
            ec = tt * 128  # edge slots this range

            # xls/xrs rows for this range (self-loop term + xr permute)
            xlr = sb.tile([128, co], EDT, name=f"xlr{l}_{r}", tag="xlr",
                          padded_shape=[128, COmax])
            nc.sync.dma_start(out=xlr[:],
                              in_=xl_loc[l][r * 128:(r + 1) * 128, :co])
            xrr = sb.tile([128, co], EDT, name=f"xrr{l}_{r}", tag="xrr",
                          padded_shape=[128, COmax])
            nc.sync.dma_start(out=xrr[:],
                              in_=xr_loc[l][r * 128:(r + 1) * 128, :co])

            # self-loop: s_self = exp(att . leaky(xl_i + xr_i))
            mself = sb.tile([128, co], EDT, name=f"ms{l}_{r}", tag="mself",
                            padded_shape=[128, COmax])
            nc.vector.tensor_tensor(out=mself[:], in0=xlr[:], in1=xrr[:],
                                    op=OP.add)
            # leaky_relu(x) = max(0.2x, x) in one DVE op
            nc.vector.scalar_tensor_tensor(
                out=mself[:], in0=mself[:], scalar=SLOPE, in1=mself[:],
                op0=OP.mult, op1=OP.max)
            nc.vector.tensor_tensor(out=mself[:], in0=mself[:],
                                    in1=wt[f"attb{l}"][:], op=OP.mult)
            lgs = sb.tile([128, 1], F32, name=f"lgs{l}_{r}", tag="lgs")
            nc.vector.tensor_reduce(out=lgs[:], in_=mself[:], axis=AX.X,
                                    op=OP.add)
            sself = sb.tile([128, 1], F32, name=f"ss{l}_{r}", tag="ss")
            nc.scalar.activation(out=sself[:], in_=lgs[:], func=ACTF.Exp)

            if tt == 0:
                # no incoming edges: softmax is all on the self-loop
                hsb = sb.tile([128, co], F32, name=f"hsb{l}_{r}", tag="hsb",
                              padded_shape=[128, COmax])
                nc.vector.tensor_tensor(out=hsb[:], in0=xlr[:],
                                        in1=wt[f"bib{l}"][:], op=OP.add)
                hsb2 = sb.tile([128, co], F32, name=f"hsb2{l}_{r}",
                               tag="hsb2", padded_shape=[128, COmax])
                nc.scalar.activation(out=hsb2[:], in_=hsb[:],
                                     func=ACTF.Relu)
                nc.sync.dma_start(out=hbuf[l][r * 128:(r + 1) * 128, :],
                                  in_=hsb2[:])
                return

            # ---- gather xl rows for this range's edges ----
            gxl = sb.tile([128, tt * cp], EDT, name=f"gxl{l}_{r}",
                          tag="gxl", bufs=3, padded_shape=[128, Tmax * CPmax])
            g3l = gxl[:].rearrange("p (t c) -> p t c", c=cp)
            dsr = sb.tile([1, ec], BF16, name=f"dsr{l}_{r}", tag="dsr",
                          padded_shape=[1, Tmax * 128])
            nc.sync.dma_start(out=dsr[:],
                              in_=dstrow_i[:, 128 * odf:128 * odf + ec])
            # HW limit: <= 1024 idxs (8 tiles) per dma_gather
            GCH = 8

            def chunked_gather(g3, t0, tn, src_ap, ix, ix0):
                for c0 in range(0, tn, GCH):
                    cn = min(GCH, tn - c0)
                    nc.gpsimd.dma_gather(
                        out_ap=g3[:, t0 + c0:t0 + c0 + cn, :],
                        in_ap=src_ap,
                        idxs_ap=ix[:, ix0 + 8 * c0:ix0 + 8 * (c0 + cn)],
                        num_idxs=128 * cn, num_idxs_reg=nreg(128 * cn),
                        elem_size=cp)

            if tlo:
                chunked_gather(g3l, 0, tlo, xlf[l][0:min(split, NFULL), :],
                               ixloC, olo)
            if thi:
                chunked_gather(g3l, tlo, thi, xlf[l][split:NFULL, :],
                               ixhiC, ohi)

            # ---- node-major one-hot wT[p, e] = (dst_e == p) ----
            wT = sb.tile([128, ec], EDT, name=f"wT{l}_{r}", tag="wT",
                         padded_shape=[128, Tmax * 128])
            NCH = 512
            for c0 in range(0, ec, NCH):
                cw = min(NCH, ec - c0)
                pdst = psum.tile([128, cw], F32, name=f"pb{l}_{r}_{c0}",
                                 tag="pt", bufs=2,
                                 padded_shape=[128, NCH])
                nc.tensor.matmul(out=pdst[:], lhsT=ones1[:],
                                 rhs=dsr[:, c0:c0 + cw], start=True,
                                 stop=True)
                nc.vector.tensor_tensor(
                    out=wT[:, c0:c0 + cw], in0=pdst[:],
                    in1=prow[:].to_broadcast([128, cw]), op=OP.is_equal)
            wT3 = wT[:].rearrange("p (t c) -> p t c", c=128)

            # ---- permute xr into edge order; m = xl_src + xr_dst ----
            mm = sb.tile([128, tt * cp], EDT, name=f"mm{l}_{r}", tag="gxr",
                         padded_shape=[128, Tmax * CPmax])
            m3 = mm[:].rearrange("p (t c) -> p t c", c=cp)
            for t in range(tt):
                pperm = psum.tile([128, co], F32, name=f"pp{l}_{r}_{t}",
                                  tag="pperm", bufs=2,
                                  padded_shape=[128, PSP])
                nc.tensor.matmul(out=pperm[:], lhsT=wT3[:, t, :],
                                 rhs=xrr[:], start=True, stop=False)
                nc.tensor.matmul(out=pperm[:], lhsT=identb[:],
                                 rhs=g3l[:, t, :co], start=False,
                                 stop=True)
                # leaky(x) = 0.6(x + (2/3)|x|); 0.6 folded into att6b
                nc.scalar.activation(out=m3[:, t, :co], in_=pperm[:],
                                     func=ACTF.Abs)
                nc.vector.scalar_tensor_tensor(
                    out=m3[:, t, :co], in0=m3[:, t, :co],
                    scalar=2.0 / 3.0, in1=pperm[:],
                    op0=OP.mult, op1=OP.add)

            v3 = m3[:, :, :co]
            att3 = wt[f"att6b{l}"][:, None, :].to_broadcast([128, tt, co])
            nc.vector.tensor_tensor(out=v3, in0=v3, in1=att3, op=OP.mult)
            logits = sb.tile([128, tt], F32, name=f"lg{l}_{r}", tag="lg",
                             padded_shape=[128, Tmax])
            nc.vector.tensor_reduce(out=logits[:], in_=v3, axis=AX.X,
                                    op=OP.add)
            s = sb.tile([128, tt], BF16, name=f"s{l}_{r}", tag="s",
                        padded_shape=[128, Tmax])
            nc.scalar.activation(out=s[:], in_=logits[:], func=ACTF.Exp)

            # ---- edge-major one-hot w3[e, t, q] = (dst_e==q) * s_e ----
            w3t = sb.tile([128, ec], EDT, name=f"w3{l}_{r}", tag="w3",
                          padded_shape=[128, Tmax * 128])
            w3 = w3t[:].rearrange("p (t c) -> p t c", c=128)
            dst3 = dstfC[:, odf:odf + tt][:, :, None].to_broadcast(
                [128, tt, 128])
            iot3 = iotab[:][:, None, :].to_broadcast([128, tt, 128])
            s3 = s[:][:, :, None].to_broadcast([128, tt, 128])
            nc.vector.tensor_tensor(out=w3, in0=dst3, in1=iot3,
                                    op=OP.is_equal)
            nc.vector.tensor_tensor(out=w3, in0=w3, in1=s3, op=OP.mult)

            # ---- scatter: pv[q,:] += s_e*xl_e ; pden[q] += s_e ----
            pv = psum.tile([128, co], F32, name=f"pv{l}_{r}", tag="pv",
                           bufs=2, padded_shape=[128, PSP])
            pden = psum.tile([128, 1], F32, name=f"pn{l}_{r}", tag="pden",
                             bufs=2)
            for t in range(tt):
                nc.tensor.matmul(out=pv[:], lhsT=w3[:, t, :],
                                 rhs=g3l[:, t, :co],
                                 start=(t == 0), stop=(t == tt - 1))
                nc.tensor.matmul(out=pden[:], lhsT=w3[:, t, :],
                                 rhs=ones[:],
                                 start=(t == 0), stop=(t == tt - 1))

            # ---- epilogue: add self-loop, normalize, bias, relu ----
            num = sb.tile([128, co], F32, name=f"nm{l}_{r}", tag="num",
                          padded_shape=[128, COmax])
            nc.vector.scalar_tensor_tensor(
                out=num[:], in0=xlr[:], scalar=sself[:, :],
                in1=pv[:], op0=OP.mult, op1=OP.add)
            den = sb.tile([128, 1], F32, name=f"den{l}_{r}", tag="den")
            nc.vector.tensor_tensor(out=den[:], in0=pden[:],
                                    in1=sself[:], op=OP.add)
            rden = sb.tile([128, 1], F32, name=f"rden{l}_{r}", tag="rden")
            nc.vector.reciprocal(out=rden[:], in_=den[:])
            hsb = sb.tile([128, co], F32, name=f"hsb{l}_{r}", tag="hsb",
                          padded_shape=[128, COmax])
            nc.vector.scalar_tensor_tensor(
                out=hsb[:], in0=num[:], scalar=rden[:, :],
                in1=wt[f"bib{l}"][:], op0=OP.mult, op1=OP.add)
            hsb2 = sb.tile([128, co], F32, name=f"hsb2{l}_{r}", tag="hsb2",
                           padded_shape=[128, COmax])
            nc.scalar.activation(out=hsb2[:], in_=hsb[:], func=ACTF.Relu)
            nc.sync.dma_start(out=hbuf[l][r * 128:(r + 1) * 128, :],
                              in_=hsb2[:])

        # ---------------- layers ----------------
        for r in range(R):
            node_range(0, r)
        for l in range(n_layers):
            nc.gpsimd.collective_compute(
                "AllGather", OP.bypass, replica_groups=rg,
                ins=[xl_loc[l][:, :]], outs=[xlf[l][:, :]])
            for r in range(R):
                edge_range(l, r)
            if l + 1 < n_layers:
                for r in range(R):
                    node_range(l + 1, r)

        # ---------------- pooling ----------------
        pg = psum.tile([128, nhid], F32, name="pg", tag="pv", bufs=2,
                       padded_shape=[128, PSP])
        for r in range(R):
            h3t = sb.tile([128, nhid], F32, name=f"h3t{r}", tag="h3t")
            nc.sync.dma_start(out=h3t[:], in_=hbuf[-1][r * 128:(r + 1) * 128, :])
            wp = sb.tile([128, 128], F32, name=f"wp{r}", tag="wpool")
            nc.vector.scalar_tensor_tensor(
                out=wp[:], in0=iota[:], scalar=batchf[:, r:r + 1],
                in1=rcntn[:, r:r + 1].to_broadcast([128, 128]),
                op0=OP.is_equal, op1=OP.mult)
            nc.tensor.matmul(out=pg[:], lhsT=wp[:], rhs=h3t[:],
                             start=(r == 0), stop=(r == R - 1))
        gsb = sb.tile([128, nhid], F32, name="gsb", tag="gsb")
        nc.vector.tensor_copy(out=gsb[:], in_=pg[:])

        # ---------------- head ----------------
        gTs = []
        for k in range(KH):
            cw = min(128, nhid - k * 128)
            ptk = psum.tile([cw, 128], F32, name=f"ptg{k}", tag="pt", bufs=2,
                            padded_shape=[128, 512])
            nc.tensor.transpose(out=ptk[:], in_=gsb[:, k * 128:k * 128 + cw],
                                identity=ident[:])
            gT = sb.tile([128, gpc], F32, name=f"gT{k}", tag=f"gT{k}")
            nc.vector.tensor_copy(out=gT[:cw, :], in_=ptk[:cw, :gpc])
            gTs.append((gT, cw))
        pz = psum.tile([gpc, nhid], F32, name="pz", tag="pv", bufs=2,
                       padded_shape=[128, PSP])
        for k in range(KH):
            gT, cw = gTs[k]
            nc.tensor.matmul(out=pz[:], lhsT=gT[:cw, :], rhs=fc1c[k][:cw, :],
                             start=(k == 0), stop=(k == KH - 1))
        zsb = sb.tile([gpc, nhid], F32, name="zsb", tag="zsb")
        nc.vector.tensor_tensor(out=zsb[:], in0=pz[:], in1=b1b[:gpc, :],
                                op=OP.add)
        zsb2 = sb.tile([gpc, nhid], F32, name="zsb2", tag="zsb2")
        nc.scalar.activation(out=zsb2[:], in_=zsb[:], func=ACTF.Relu)
        zTs = []
        for k in range(KH):
            cw = min(128, nhid - k * 128)
            ptk = psum.tile([cw, gpc], F32, name=f"ptz{k}", tag="pt", bufs=2,
                            padded_shape=[128, 512])
            nc.tensor.transpose(out=ptk[:], in_=zsb2[:, k * 128:k * 128 + cw],
                                identity=ident[:gpc, :gpc])
            zT = sb.tile([128, gpc], F32, name=f"zT{k}", tag=f"zT{k}")
            nc.vector.tensor_copy(out=zT[:cw, :], in_=ptk[:cw, :])
            zTs.append((zT, cw))
        osb = sb.tile([gpc, nout], F32, name="osb", tag="osb")
        for j in range(nsplit):
            po = psum.tile([gpc, nw], F32, name=f"po{j}", tag="pperm", bufs=2,
                           padded_shape=[128, PSP])
            for k in range(KH):
                zT, cw = zTs[k]
                nc.tensor.matmul(out=po[:], lhsT=zT[:cw, :],
                                 rhs=fc2c[k][:cw, j * nw:(j + 1) * nw],
                                 start=(k == 0), stop=(k == KH - 1))
            nc.vector.tensor_tensor(out=osb[:, j * nw:(j + 1) * nw], in0=po[:],
                                    in1=b2b[:gpc, j * nw:(j + 1) * nw],
                                    op=OP.add)
        nc.sync.dma_start(out=out_t[:, :], in_=osb[:])

    nc.compile()
    return nc


# ----------------------------------------------------------------------------
# Entry point
# ----------------------------------------------------------------------------

def _augment(inputs):
    inputs = dict(inputs)
    inputs["_dims"] = [(300, 128), (128, 128), (128, 128), (128, 300)]
    inputs["_nhid"] = 300
    inputs["_nout"] = 768
    inputs["_n_graphs"] = 256
    return inputs


def run(inputs, trace=False, n_cores=8):
    inputs = _augment(inputs)
    meta, in_maps = preprocess(inputs, n_cores=n_cores)
    nc = build_nc(meta)
    res = run_bass_kernel_spmd(nc, in_maps, core_ids=list(range(n_cores)),
                               trace=trace)
    out = np.concatenate([r["out"] for r in res.results], axis=0)
    return out, res


def kernel(**inputs):
    out, _ = run(inputs, trace=False)
    return out


# revision 24
# speedup vs baseline: 1.3657x; 1.0800x over previous
"""GATv2 AttentionEncoder kernel for Trainium2 (8 NeuronCores, Bass/Tile).

Strategy (sharding_hint: shard by graph):
  - 256 graphs -> 8 cores x 32 graphs. Each core owns a contiguous,
    graph-aligned node slice, padded to NMAXP rows (multiple of 128).
  - Per layer: node-phase matmuls (xl = h@Wl+bl, xr = h@Wr+br) run on the
    local slice; xl is AllGathered (src edges reference any node), xr stays
    local (edges are bucketed by dst core).
  - Edge phase: real edges only (self-loops folded analytically in the
    epilogue), sorted by dst 128-node range. Per range: dma_gather xl rows
    for the range's edge tiles (int16 idx, lo/hi window split for the 32k
    limit). xr values are NOT gathered: a node-major one-hot wT (built via a
    K=1 outer-product broadcast of dst ids + DVE is_equal) permutes the
    range's xr tile into edge order with one PE matmul per 128-edge tile.
    Batched DVE ops compute leaky_relu/att-dot/exp, a batched edge-major
    one-hot w3 (alpha folded) scatters alpha*xl into PSUM via per-tile
    matmuls (plus rhs=ones matmuls for the softmax denominators); the
    self-loop term (exp(att.leaky(xl_i+xr_i)), xl_i) is added on the node
    layout before normalization.
  - Pooling is graph-local (one-hot matmul with 1/cnt folded in), MLP head
    computed per-core on its 32 graphs; host concatenates.
"""

import sys

sys.path.insert(0, "/opt/trn_rl_repo")

import contextlib

import ml_dtypes
import numpy as np

import concourse.bass as bass
import concourse.bacc as bacc
import concourse.mybir as mybir
import concourse.tile as tile
from concourse.bass_utils import run_bass_kernel_spmd

F32 = mybir.dt.float32
BF16 = mybir.dt.bfloat16
I16 = mybir.dt.int16
AX = mybir.AxisListType
OP = mybir.AluOpType
ACTF = mybir.ActivationFunctionType

SLOPE = 0.2


# ----------------------------------------------------------------------------
# Host-side preprocessing
# ----------------------------------------------------------------------------

def _wrap_idx(arr):
    """[n] int array (n % 16 == 0) -> [128, n/16] int16, slot i at
    [i%16, i//16], replicated 8x across partition groups of 16."""
    n = len(arr)
    w = np.ascontiguousarray(arr.reshape(n // 16, 16).T).astype(np.int16)
    return np.tile(w, (8, 1))


def preprocess(inputs, n_cores=8, split=32768):
    x = np.asarray(inputs["x"], np.float32)
    ei = np.asarray(inputs["edge_index"], np.int64)
    batch = np.asarray(inputs["batch"], np.int64)
    N, n_in = x.shape
    G = inputs["_n_graphs"]
    gpc = G // n_cores
    dims = inputs["_dims"]
    ebytes = 2
    # row byte-stride of xl/xr must be a multiple of 256B
    cpads = [(co * ebytes + 255) // 256 * 256 // ebytes for (_, co) in dims]

    cnt = np.bincount(batch, minlength=G)
    gs = np.add.reduceat(cnt, np.arange(0, G, gpc))  # nodes per core
    bounds = np.concatenate([[0], np.cumsum(gs)]).astype(np.int64)
    NMAXP = int((gs.max() + 127) // 128 * 128)
    R = NMAXP // 128
    NFULL = n_cores * NMAXP
    assert NFULL <= split * 2, (NFULL, split)

    # remap node ids into the padded global layout
    newid = np.empty(N, np.int64)
    for r in range(n_cores):
        n0, n1 = bounds[r], bounds[r + 1]
        newid[n0:n1] = NMAXP * r + np.arange(n1 - n0)

    # real edges only; self-loop contributions are computed analytically
    src = newid[ei[0]]
    dst = newid[ei[1]]

    # bucket edges per (core, range, lo/hi); order within a bucket irrelevant
    core_of = dst // NMAXP
    dstl = dst - core_of * NMAXP
    rng_of = dstl // 128
    is_hi = (src >= split).astype(np.int64)
    key = (core_of * R + rng_of) * 2 + is_hi
    order = np.argsort(key, kind="stable")
    src_s, dstl_s, key_s = src[order], dstl[order], key[order]
    uniq, starts = np.unique(key_s, return_index=True)
    starts = list(starts) + [len(key_s)]
    lo_lists = [[None] * R for _ in range(n_cores)]
    hi_lists = [[None] * R for _ in range(n_cores)]
    for i, k in enumerate(uniq):
        e0, e1 = starts[i], starts[i + 1]
        c, rem = divmod(int(k), 2 * R)
        g, h = divmod(rem, 2)
        pair = (src_s[e0:e1], dstl_s[e0:e1])
        (hi_lists if h else lo_lists)[c][g] = pair

    empty = (np.zeros(0, np.int64), np.zeros(0, np.int64))
    TLO = np.zeros(R, np.int64)
    THI = np.zeros(R, np.int64)
    for g in range(R):
        for c in range(n_cores):
            lo = lo_lists[c][g] or empty
            hi = hi_lists[c][g] or empty
            TLO[g] = max(TLO[g], -(-len(lo[0]) // 128))
            THI[g] = max(THI[g], -(-len(hi[0]) // 128))
    T = TLO + THI

    meta = dict(
        n_cores=n_cores, gpc=gpc, G=G, NMAXP=NMAXP, R=R, NFULL=NFULL,
        split=split, TLO=TLO.tolist(), THI=THI.tolist(), T=T.tolist(),
        dims=dims, cpads=cpads, n_in=n_in,
        nhid=inputs["_nhid"], nout=inputs["_nout"],
    )

    # ---- shared const arrays ----
    iota = np.tile(np.arange(128, dtype=np.float32), (128, 1))
    ident = np.eye(128, dtype=np.float32)
    ones = np.ones((128, 1), np.float32).astype(ml_dtypes.bfloat16)

    def bc(v, w):  # broadcast a [w] vector to [128, w]
        return np.tile(np.asarray(v, np.float32).reshape(1, w), (128, 1))

    def padk(w):  # pad leading dim to a multiple of 128
        k = (-(-w.shape[0] // 128)) * 128
        out = np.zeros((k,) + w.shape[1:], np.float32)
        out[: w.shape[0]] = w
        return out

    consts = dict(iota=iota, ident=ident, ones=ones,
                  identb=ident.astype(ml_dtypes.bfloat16))
    for l, (ci, co) in enumerate(dims):
        consts[f"wl{l}"] = padk(np.asarray(inputs[f"Wl{l}"], np.float32))
        consts[f"wr{l}"] = padk(np.asarray(inputs[f"Wr{l}"], np.float32))
        consts[f"blb{l}"] = bc(inputs[f"bl{l}"], co)
        consts[f"brb{l}"] = bc(inputs[f"br{l}"], co)
        consts[f"bib{l}"] = bc(inputs[f"bias{l}"], co)
        consts[f"attb{l}"] = bc(inputs[f"att{l}"], co).astype(ml_dtypes.bfloat16)
        # edge path folds leaky's 0.6 factor into att: leaky(x)=0.6(x+(2/3)|x|)
        consts[f"att6b{l}"] = (0.6 * bc(inputs[f"att{l}"], co)).astype(
            ml_dtypes.bfloat16)
    consts["fc1"] = padk(np.asarray(inputs["fc1_W"], np.float32))
    consts["fc2"] = padk(np.asarray(inputs["fc2_W"], np.float32))
    consts["b1b"] = bc(inputs["fc1_b"], meta["nhid"])
    consts["b2b"] = bc(inputs["fc2_b"], meta["nout"])

    rcnt = 1.0 / np.maximum(cnt, 1).astype(np.float64)
    KIN = -(-n_in // 128)

    in_maps = []
    for c in range(n_cores):
        n0, n1 = bounds[c], bounds[c + 1]
        nl = int(n1 - n0)
        xT = np.zeros((KIN * 128, NMAXP), np.float32)
        xT[:n_in, :nl] = x[n0:n1].T
        ilo, ihi, w3b, wTb = [], [], [], []
        for g in range(R):
            lo = lo_lists[c][g] or empty
            hi = hi_lists[c][g] or empty
            nlo, nhi = 128 * int(TLO[g]), 128 * int(THI[g])
            sl = np.zeros(nlo, np.int64)
            sl[: len(lo[0])] = lo[0]
            sh = np.zeros(nhi, np.int64)
            sh[: len(hi[0])] = hi[0] - split
            df = np.full(nlo + nhi, -1, np.int64)
            df[: len(lo[1])] = lo[1] - g * 128
            df[nlo: nlo + len(hi[1])] = hi[1] - g * 128
            if nlo:
                ilo.append(_wrap_idx(sl))
            if nhi:
                ihi.append(_wrap_idx(sh))
            if nlo + nhi:
                ntt = (nlo + nhi) // 128
                dfm = df.reshape(ntt, 128)  # [t, edge slot]
                tq, pq = np.nonzero(dfm >= 0)
                w3 = np.zeros((128, ntt, 128), ml_dtypes.bfloat16)
                w3[pq, tq, dfm[tq, pq]] = 1  # [edge p, t, node q]
                wT = np.zeros((128, ntt, 128), ml_dtypes.bfloat16)
                wT[dfm[tq, pq], tq, pq] = 1  # [node p, t, edge e]
                w3b.append(w3.reshape(128, ntt * 128))
                wTb.append(wT.reshape(128, ntt * 128))
        m = dict(
            xT=xT,
            idx_lo=np.concatenate(ilo, 1) if ilo else np.zeros((128, 0), np.int16),
            idx_hi=np.concatenate(ihi, 1) if ihi else np.zeros((128, 0), np.int16),
            w3h=np.concatenate(w3b, 1)
            if w3b else np.zeros((128, 0), ml_dtypes.bfloat16),
            wTh=np.concatenate(wTb, 1)
            if wTb else np.zeros((128, 0), ml_dtypes.bfloat16),
        )
        bf = np.full(NMAXP, -1.0, np.float32)
        rc = np.zeros(NMAXP, np.float32)
        bf[:nl] = (batch[n0:n1] - c * gpc).astype(np.float32)
        rc[:nl] = rcnt[batch[n0:n1]].astype(np.float32)
        m["batchf"] = np.ascontiguousarray(bf.reshape(R, 128).T)
        m["rcntn"] = np.ascontiguousarray(rc.reshape(R, 128).T)
        m.update(consts)
        in_maps.append({k: np.ascontiguousarray(v) for k, v in m.items()})

    return meta, in_maps


# ----------------------------------------------------------------------------
# Bass program
# ----------------------------------------------------------------------------

def build_nc(meta):
    n_cores = meta["n_cores"]
    NMAXP, R, NFULL = meta["NMAXP"], meta["R"], meta["NFULL"]
    split = meta["split"]
    TLO, THI, T = meta["TLO"], meta["THI"], meta["T"]
    dims, cpads = meta["dims"], meta["cpads"]
    n_in, nhid, nout, gpc = meta["n_in"], meta["nhid"], meta["nout"], meta["gpc"]
    EDT = BF16
    KIN = -(-n_in // 128)
    KH = -(-nhid // 128)
    n_layers = len(dims)
    Tmax = max(T)
    CPmax = max(cpads)
    COmax = max(co for _, co in dims)
    nsplit = -(-nout // 512)
    nw = nout // nsplit
    PSP = max(COmax, nhid, nw)  # shared psum tile width (f32, <= 1 bank)
    DCOL = (COmax + 31) // 32 * 32  # aligned denominator column in pv
    assert (PSP + 1) * 4 <= 2048 and DCOL < PSP
    rg = [list(range(n_cores))]

    nc = bacc.Bacc(trn_type="TRN2", num_devices=n_cores)

    def inp(name, shape, dtype=F32):
        return nc.dram_tensor(name, list(shape), dtype, kind="ExternalInput").ap()

    xT = inp("xT", [KIN * 128, NMAXP])
    idx_lo = inp("idx_lo", [128, max(8 * sum(TLO), 1)], I16)
    idx_hi = inp("idx_hi", [128, max(8 * sum(THI), 1)], I16)
    w3h_i = inp("w3h", [128, max(128 * sum(T), 1)], BF16)
    wTh_i = inp("wTh", [128, max(128 * sum(T), 1)], BF16)
    batchf_i = inp("batchf", [128, R])
    rcntn_i = inp("rcntn", [128, R])
    iota_i = inp("iota", [128, 128])
    ident_i = inp("ident", [128, 128])
    identb_i = inp("identb", [128, 128], BF16)
    ones_i = inp("ones", [128, 1], BF16)
    w_i = {}
    for l, (ci, co) in enumerate(dims):
        kc = -(-ci // 128)
        w_i[f"wl{l}"] = inp(f"wl{l}", [kc * 128, co])
        w_i[f"wr{l}"] = inp(f"wr{l}", [kc * 128, co])
        for nm in ("blb", "brb", "bib"):
            w_i[f"{nm}{l}"] = inp(f"{nm}{l}", [128, co])
        w_i[f"attb{l}"] = inp(f"attb{l}", [128, co], BF16)
        w_i[f"att6b{l}"] = inp(f"att6b{l}", [128, co], BF16)
    fc1_i = inp("fc1", [KH * 128, nhid])
    fc2_i = inp("fc2", [KH * 128, nout])
    b1b_i = inp("b1b", [128, nhid])
    b2b_i = inp("b2b", [128, nout])
    out_t = nc.dram_tensor("out", [gpc, nout], F32, kind="ExternalOutput").ap()

    with tile.TileContext(nc) as tc, contextlib.ExitStack() as ctx:
        cpool = ctx.enter_context(tc.tile_pool(name="consts", bufs=1))
        sb = ctx.enter_context(tc.tile_pool(name="sb", bufs=2))
        psum = ctx.enter_context(tc.tile_pool(name="ps", bufs=1, space="PSUM"))
        dram = ctx.enter_context(tc.tile_pool(name="dr", bufs=1, space="DRAM"))

        def cload(ap, name, rows=None):
            shape = list(ap.shape) if rows is None else [rows, ap.shape[1]]
            t = cpool.tile(shape, ap.dtype, name=name, tag=name)
            nc.sync.dma_start(out=t[:], in_=ap if rows is None else ap[:rows, :])
            return t

        iota = cload(iota_i, "iota")
        ident = cload(ident_i, "ident")
        identb = cload(identb_i, "identb")
        ones = cload(ones_i, "ones")
        wt = {}
        for l, (ci, co) in enumerate(dims):
            kc = -(-ci // 128)
            for side in ("wl", "wr"):
                for k in range(kc):
                    nm = f"{side}{l}k{k}"
                    t = cpool.tile([128, co], F32, name=nm, tag=nm)
                    nc.sync.dma_start(
                        out=t[:], in_=w_i[f"{side}{l}"][k * 128:(k + 1) * 128, :])
                    wt[nm] = t
            for nm0 in ("blb", "brb", "bib", "attb", "att6b"):
                wt[f"{nm0}{l}"] = cload(w_i[f"{nm0}{l}"], f"{nm0}{l}")
        fc1c, fc2c = [], []
        for k in range(KH):
            t = cpool.tile([128, nhid], F32, name=f"fc1k{k}", tag=f"fc1k{k}")
            nc.sync.dma_start(out=t[:], in_=fc1_i[k * 128:(k + 1) * 128, :])
            fc1c.append(t)
            t = cpool.tile([128, nout], F32, name=f"fc2k{k}", tag=f"fc2k{k}")
            nc.sync.dma_start(out=t[:], in_=fc2_i[k * 128:(k + 1) * 128, :])
            fc2c.append(t)
        b1b = cload(b1b_i, "b1b")
        b2b = cload(b2b_i, "b2b")
        batchf = cload(batchf_i, "batchf")
        rcntn = cload(rcntn_i, "rcntn")
        # graph topology is layer-invariant: load idx/dst arrays once
        ixloC = cload(idx_lo, "ixloC")
        ixhiC = cload(idx_hi, "ixhiC")

        # persistent DRAM buffers; AllGather outputs are distinct per layer
        # (a fast core's AG for layer l+1 may write a slow core's output
        # buffer while it still reads layer l's), and Shared for perf.
        xlf_space = "Shared" if n_cores > 4 else "Local"
        xlf = [dram.tile([NFULL, cpads[l]], EDT, name=f"xlf{l}", tag=f"xlf{l}",
                         addr_space=xlf_space) for l in range(n_layers)]
        xl_loc = [dram.tile([NMAXP, cpads[l]], EDT, name=f"xlloc{l}",
                            tag=f"xlloc{l}") for l in range(n_layers)]
        xr_loc = [dram.tile([NMAXP, cpads[l]], EDT, name=f"xrloc{l}",
                            tag=f"xrloc{l}") for l in range(n_layers)]
        hbuf = [dram.tile([NMAXP, dims[l][1]], F32, name=f"h{l}", tag=f"h{l}")
                for l in range(n_layers)]

        reg_cache = {}

        def nreg(v):
            if v not in reg_cache:
                reg_cache[v] = nc.gpsimd.to_reg(v)
            return reg_cache[v]

        # prefix offsets into the per-range packed arrays
        OLO, OHI, ODF = [], [], []
        olo = ohi = odf = 0
        for r in range(R):
            OLO.append(olo); OHI.append(ohi); ODF.append(odf)
            olo += 8 * TLO[r]; ohi += 8 * THI[r]; odf += T[r]

        def node_range(l, r):
            ci, co = dims[l]
            kc = -(-ci // 128)
            hTs = []
            if l == 0:
                for k in range(kc):
                    hT = sb.tile([128, 128], F32, name=f"hT{l}_{r}_{k}",
                                 tag=f"hT{k}")
                    nc.sync.dma_start(
                        out=hT[:],
                        in_=xT[k * 128:(k + 1) * 128, r * 128:(r + 1) * 128])
                    hTs.append(hT)
            else:
                ht = sb.tile([128, ci], F32, name=f"ht{l}_{r}", tag="ht",
                             padded_shape=[128, 128])
                nc.sync.dma_start(
                    out=ht[:], in_=hbuf[l - 1][r * 128:(r + 1) * 128, :])
                pt = psum.tile([ci, 128], F32, name=f"pt{l}_{r}", tag="pt",
                               bufs=1, padded_shape=[128, 512])
                nc.tensor.transpose(out=pt[:], in_=ht[:], identity=ident[:])
                hT = sb.tile([128, 128], F32, name=f"hT{l}_{r}", tag="hT0")
                nc.vector.tensor_copy(out=hT[:ci, :], in_=pt[:])
                hTs.append(hT)
            krows = [128] * kc if l == 0 else [ci]
            pxl = psum.tile([128, co], F32, name=f"pxl{l}_{r}", tag="pnode",
                            bufs=2, padded_shape=[128, PSP])
            pxr = psum.tile([128, co], F32, name=f"pxr{l}_{r}", tag="pnode",
                            bufs=2, padded_shape=[128, PSP])
            for k in range(kc):
                nc.tensor.matmul(out=pxl[:], lhsT=hTs[k][:krows[k], :],
                                 rhs=wt[f"wl{l}k{k}"][:krows[k], :],
                                 start=(k == 0), stop=(k == kc - 1))
            for k in range(kc):
                nc.tensor.matmul(out=pxr[:], lhsT=hTs[k][:krows[k], :],
                                 rhs=wt[f"wr{l}k{k}"][:krows[k], :],
                                 start=(k == 0), stop=(k == kc - 1))
            xls = sb.tile([128, co], EDT, name=f"xls{l}_{r}", tag="xls",
                          padded_shape=[128, COmax])
            xrs = sb.tile([128, co], EDT, name=f"xrs{l}_{r}", tag="xrs",
                          padded_shape=[128, COmax])
            nc.vector.tensor_tensor(out=xls[:], in0=pxl[:],
                                    in1=wt[f"blb{l}"][:], op=OP.add)
            nc.vector.tensor_tensor(out=xrs[:], in0=pxr[:],
                                    in1=wt[f"brb{l}"][:], op=OP.add)
            nc.sync.dma_start(out=xl_loc[l][r * 128:(r + 1) * 128, :co],
                              in_=xls[:])
            nc.sync.dma_start(out=xr_loc[l][r * 128:(r + 1) * 128, :co],
                              in_=xrs[:])

        def edge_range(l, r):
            ci, co = dims[l]
            cp = cpads[l]
            tlo, thi, tt = TLO[r], THI[r], T[r]
            olo, ohi, odf = OLO[r], OHI[r], ODF[r]
            ec = tt * 128  # edge slots this range

            # xls/xrs rows for this range (self-loop term + xr permute)
            xlr = sb.tile([128, co], EDT, name=f"xlr{l}_{r}", tag="xlr",
                          padded_shape=[128, COmax])
            nc.sync.dma_start(out=xlr[:],
                              in_=xl_loc[l][r * 128:(r + 1) * 128, :co])
            xrr = sb.tile([128, co], EDT, name=f"xrr{l}_{r}", tag="xrr",
                          padded_shape=[128, COmax])
            nc.sync.dma_start(out=xrr[:],
                              in_=xr_loc[l][r * 128:(r + 1) * 128, :co])

            # self-loop: s_self = exp(att . leaky(xl_i + xr_i))
            mself = sb.tile([128, co], EDT, name=f"ms{l}_{r}", tag="mself",
                            padded_shape=[128, COmax])
            nc.vector.tensor_tensor(out=mself[:], in0=xlr[:], in1=xrr[:],
                                    op=OP.add)
            # leaky_relu(x) = max(0.2x, x) in one DVE op
            nc.vector.scalar_tensor_tensor(
                out=mself[:], in0=mself[:], scalar=SLOPE, in1=mself[:],
                op0=OP.mult, op1=OP.max)
            nc.vector.tensor_tensor(out=mself[:], in0=mself[:],
                                    in1=wt[f"attb{l}"][:], op=OP.mult)
            lgs = sb.tile([128, 1], F32, name=f"lgs{l}_{r}", tag="lgs")
            nc.vector.tensor_reduce(out=lgs[:], in_=mself[:], axis=AX.X,
                                    op=OP.add)
            sself = sb.tile([128, 1], F32, name=f"ss{l}_{r}", tag="ss")
            nc.scalar.activation(out=sself[:], in_=lgs[:], func=ACTF.Exp)

            if tt == 0:
                # no incoming edges: softmax is all on the self-loop
                hsb = sb.tile([128, co], F32, name=f"hsb{l}_{r}", tag="hsb",
                              padded_shape=[128, COmax])
                nc.vector.tensor_tensor(out=hsb[:], in0=xlr[:],
                                        in1=wt[f"bib{l}"][:], op=OP.add)
                hsb2 = sb.tile([128, co], F32, name=f"hsb2{l}_{r}",
                               tag="hsb2", padded_shape=[128, COmax])
                nc.scalar.activation(out=hsb2[:], in_=hsb[:],
                                     func=ACTF.Relu)
                nc.sync.dma_start(out=hbuf[l][r * 128:(r + 1) * 128, :],
                                  in_=hsb2[:])
                return

            # ---- gather xl rows for this range's edges ----
            gxl = sb.tile([128, tt * cp], EDT, name=f"gxl{l}_{r}",
                          tag="gxl", bufs=4, padded_shape=[128, Tmax * CPmax])
            g3l = gxl[:].rearrange("p (t c) -> p t c", c=cp)
            # host-precomputed one-hots (layer-invariant)
            wTt = sb.tile([128, ec], EDT, name=f"wT{l}_{r}", tag="wT",
                          padded_shape=[128, Tmax * 128])
            nc.sync.dma_start(out=wTt[:],
                              in_=wTh_i[:, 128 * odf:128 * odf + ec])
            wT3 = wTt[:].rearrange("p (t c) -> p t c", c=128)
            w3t = sb.tile([128, ec], EDT, name=f"w3{l}_{r}", tag="w3",
                          padded_shape=[128, Tmax * 128])
            nc.sync.dma_start(out=w3t[:],
                              in_=w3h_i[:, 128 * odf:128 * odf + ec])
            w3 = w3t[:].rearrange("p (t c) -> p t c", c=128)
            # HW limit: <= 1024 idxs (8 tiles) per dma_gather
            GCH = 8

            def chunked_gather(g3, t0, tn, src_ap, ix, ix0):
                for c0 in range(0, tn, GCH):
                    cn = min(GCH, tn - c0)
                    nc.gpsimd.dma_gather(
                        out_ap=g3[:, t0 + c0:t0 + c0 + cn, :],
                        in_ap=src_ap,
                        idxs_ap=ix[:, ix0 + 8 * c0:ix0 + 8 * (c0 + cn)],
                        num_idxs=128 * cn, num_idxs_reg=nreg(128 * cn),
                        elem_size=cp)

            if tlo:
                chunked_gather(g3l, 0, tlo, xlf[l][0:min(split, NFULL), :],
                               ixloC, olo)
            if thi:
                chunked_gather(g3l, tlo, thi, xlf[l][split:NFULL, :],
                               ixhiC, ohi)

            # ---- permute xr into edge order; m = xl_src + xr_dst ----
            mm = sb.tile([128, tt * cp], EDT, name=f"mm{l}_{r}", tag="gxr",
                         bufs=3, padded_shape=[128, Tmax * CPmax])
            m3 = mm[:].rearrange("p (t c) -> p t c", c=cp)
            for t in range(tt):
                pperm = psum.tile([128, co], F32, name=f"pp{l}_{r}_{t}",
                                  tag="pperm", bufs=2,
                                  padded_shape=[128, PSP])
                nc.tensor.matmul(out=pperm[:], lhsT=wT3[:, t, :],
                                 rhs=xrr[:], start=True, stop=False)
                nc.tensor.matmul(out=pperm[:], lhsT=identb[:],
                                 rhs=g3l[:, t, :co], start=False,
                                 stop=True)
                # leaky(x) = 0.6(x + (2/3)|x|); 0.6 folded into att6b
                nc.scalar.activation(out=m3[:, t, :co], in_=pperm[:],
                                     func=ACTF.Abs)
                nc.vector.scalar_tensor_tensor(
                    out=m3[:, t, :co], in0=m3[:, t, :co],
                    scalar=2.0 / 3.0, in1=pperm[:],
                    op0=OP.mult, op1=OP.add)

            v3 = m3[:, :, :co]
            att3 = wt[f"att6b{l}"][:, None, :].to_broadcast([128, tt, co])
            nc.vector.tensor_tensor(out=v3, in0=v3, in1=att3, op=OP.mult)
            logits = sb.tile([128, tt], F32, name=f"lg{l}_{r}", tag="lg",
                             padded_shape=[128, Tmax])
            nc.vector.tensor_reduce(out=logits[:], in_=v3, axis=AX.X,
                                    op=OP.add)
            s = sb.tile([128, tt], BF16, name=f"s{l}_{r}", tag="s",
                        padded_shape=[128, Tmax])
            nc.scalar.activation(out=s[:], in_=logits[:], func=ACTF.Exp)

            # ---- fold alpha into the loaded edge-major one-hot ----
            s3 = s[:][:, :, None].to_broadcast([128, tt, 128])
            nc.vector.tensor_tensor(out=w3, in0=w3, in1=s3, op=OP.mult)

            # ---- scatter: pv[q,:] += s_e*xl_e ; pden[q] += s_e ----
            pv = psum.tile([128, co], F32, name=f"pv{l}_{r}", tag="pv",
                           bufs=2, padded_shape=[128, PSP])
            pden = psum.tile([128, 1], F32, name=f"pn{l}_{r}", tag="pden",
                             bufs=1)
            for t in range(tt):
                nc.tensor.matmul(out=pv[:], lhsT=w3[:, t, :],
                                 rhs=g3l[:, t, :co],
                                 start=(t == 0), stop=(t == tt - 1))
                nc.tensor.matmul(out=pden[:], lhsT=w3[:, t, :],
                                 rhs=ones[:],
                                 start=(t == 0), stop=(t == tt - 1))

            # ---- epilogue: add self-loop, normalize, bias, relu ----
            num = sb.tile([128, co], F32, name=f"nm{l}_{r}", tag="num",
                          padded_shape=[128, COmax])
            nc.vector.scalar_tensor_tensor(
                out=num[:], in0=xlr[:], scalar=sself[:, :],
                in1=pv[:], op0=OP.mult, op1=OP.add)
            den = sb.tile([128, 1], F32, name=f"den{l}_{r}", tag="den")
            nc.vector.tensor_tensor(out=den[:], in0=pden[:],
                                    in1=sself[:], op=OP.add)
            rden = sb.tile([128, 1], F32, name=f"rden{l}_{r}", tag="rden")
            nc.vector.reciprocal(out=rden[:], in_=den[:])
            hsb = sb.tile([128, co], F32, name=f"hsb{l}_{r}", tag="hsb",
                          padded_shape=[128, COmax])
            nc.vector.scalar_tensor_tensor(
                out=hsb[:], in0=num[:], scalar=rden[:, :],
                in1=wt[f"bib{l}"][:], op0=OP.mult, op1=OP.add)
            hsb2 = sb.tile([128, co], F32, name=f"hsb2{l}_{r}", tag="hsb2",
                           padded_shape=[128, COmax])
            nc.scalar.activation(out=hsb2[:], in_=hsb[:], func=ACTF.Relu)
            nc.sync.dma_start(out=hbuf[l][r * 128:(r + 1) * 128, :],
                              in_=hsb2[:])

        # ---------------- layers ----------------
        # node(l+1) interleaves into edge(l) (own psum tag) so the next
        # AllGather fires right after the last edge range of layer l.
        for r in range(R):
            node_range(0, r)
        for l in range(n_layers):
            nc.gpsimd.collective_compute(
                "AllGather", OP.bypass, replica_groups=rg,
                ins=[xl_loc[l][:, :]], outs=[xlf[l][:, :]])
            for r in range(R):
                edge_range(l, r)
                if l + 1 < n_layers:
                    node_range(l + 1, r)

        # ---------------- pooling ----------------
        pg = psum.tile([128, nhid], F32, name="pg", tag="pv", bufs=2,
                       padded_shape=[128, PSP])
        for r in range(R):
            h3t = sb.tile([128, nhid], F32, name=f"h3t{r}", tag="h3t")
            nc.sync.dma_start(out=h3t[:], in_=hbuf[-1][r * 128:(r + 1) * 128, :])
            wp = sb.tile([128, 128], F32, name=f"wp{r}", tag="wpool")
            nc.vector.scalar_tensor_tensor(
                out=wp[:], in0=iota[:], scalar=batchf[:, r:r + 1],
                in1=rcntn[:, r:r + 1].to_broadcast([128, 128]),
                op0=OP.is_equal, op1=OP.mult)
            nc.tensor.matmul(out=pg[:], lhsT=wp[:], rhs=h3t[:],
                             start=(r == 0), stop=(r == R - 1))
        gsb = sb.tile([128, nhid], F32, name="gsb", tag="gsb")
        nc.vector.tensor_copy(out=gsb[:], in_=pg[:])

        # ---------------- head ----------------
        gTs = []
        for k in range(KH):
            cw = min(128, nhid - k * 128)
            ptk = psum.tile([cw, 128], F32, name=f"ptg{k}", tag="pt", bufs=1,
                            padded_shape=[128, 512])
            nc.tensor.transpose(out=ptk[:], in_=gsb[:, k * 128:k * 128 + cw],
                                identity=ident[:])
            gT = sb.tile([128, gpc], F32, name=f"gT{k}", tag=f"gT{k}")
            nc.vector.tensor_copy(out=gT[:cw, :], in_=ptk[:cw, :gpc])
            gTs.append((gT, cw))
        pz = psum.tile([gpc, nhid], F32, name="pz", tag="pv", bufs=2,
                       padded_shape=[128, PSP])
        for k in range(KH):
            gT, cw = gTs[k]
            nc.tensor.matmul(out=pz[:], lhsT=gT[:cw, :], rhs=fc1c[k][:cw, :],
                             start=(k == 0), stop=(k == KH - 1))
        zsb = sb.tile([gpc, nhid], F32, name="zsb", tag="zsb")
        nc.vector.tensor_tensor(out=zsb[:], in0=pz[:], in1=b1b[:gpc, :],
                                op=OP.add)
        zsb2 = sb.tile([gpc, nhid], F32, name="zsb2", tag="zsb2")
        nc.scalar.activation(out=zsb2[:], in_=zsb[:], func=ACTF.Relu)
        zTs = []
        for k in range(KH):
            cw = min(128, nhid - k * 128)
            ptk = psum.tile([cw, gpc], F32, name=f"ptz{k}", tag="pt", bufs=1,
                            padded_shape=[128, 512])
            nc.tensor.transpose(out=ptk[:], in_=zsb2[:, k * 128:k * 128 + cw],
                                identity=ident[:gpc, :gpc])
            zT = sb.tile([128, gpc], F32, name=f"zT{k}", tag=f"zT{k}")
            nc.vector.tensor_copy(out=zT[:cw, :], in_=ptk[:cw, :])
            zTs.append((zT, cw))
        osb = sb.tile([gpc, nout], F32, name="osb", tag="osb")
        for j in range(nsplit):
            po = psum.tile([gpc, nw], F32, name=f"po{j}", tag="pperm", bufs=2,
                           padded_shape=[128, PSP])
            for k in range(KH):
                zT, cw = zTs[k]
                nc.tensor.matmul(out=po[:], lhsT=zT[:cw, :],
                                 rhs=fc2c[k][:cw, j * nw:(j + 1) * nw],
                                 start=(k == 0), stop=(k == KH - 1))
            nc.vector.tensor_tensor(out=osb[:, j * nw:(j + 1) * nw], in0=po[:],
                                    in1=b2b[:gpc, j * nw:(j + 1) * nw],
                                    op=OP.add)
        nc.sync.dma_start(out=out_t[:, :], in_=osb[:])

    nc.compile()
    return nc


# ----------------------------------------------------------------------------
# Entry point
# ----------------------------------------------------------------------------

def _augment(inputs):
    inputs = dict(inputs)
    inputs["_dims"] = [(300, 128), (128, 128), (128, 128), (128, 300)]
    inputs["_nhid"] = 300
    inputs["_nout"] = 768
    inputs["_n_graphs"] = 256
    return inputs


def run(inputs, trace=False, n_cores=8):
    inputs = _augment(inputs)
    meta, in_maps = preprocess(inputs, n_cores=n_cores)
    nc = build_nc(meta)
    res = run_bass_kernel_spmd(nc, in_maps, core_ids=list(range(n_cores)),
                               trace=trace)
    out = np.concatenate([r["out"] for r in res.results], axis=0)
    return out, res


def kernel(**inputs):
    out, _ = run(inputs, trace=False)
    return out
